# revision 16
# baseline (speedup 1.0000x reference)
"""Trainium2 Bass kernel for nn_EnhancedGCN (GCN -> GAT -> GCN -> GCN -> mean -> linear).

Strategy (8 NeuronCores, dst-sharded message passing):
- Host: add self loops, in-degree -> dinv (compile-time), relabel nodes
  (core = n % 8, rank grouped by region r = ceil(indeg/K), K=8 slots/row,
  region sizes padded uniform across cores). Slot i = k*RPAD + row holds the
  table index of the k-th in-edge source of `row`; gathered slot i lands at
  SBUF [i%128, i//128], i.e. rows are partition-wrapped and k is a free-dim
  plane -> K-reduce is log-step DVE adds, region combine is block-offset adds.
- Aggregation passes gather from per-core HBM tables via SWDGE dma_gather
  (chunks of 1024 idxs, 4 queues). GCN tables: [TABN+8, 64] f32 rows (16
  used). GAT table: [TABN+8, 128] f32-word rows = [a_src 12 f32 | pad |
  hh 192 bf16 | pad]; the pad row has a_src=-200 so exp() ~ 0.
- a_src/a_dst attention vectors are folded into Wg on the host (Was/Wad);
  softmax max-subtraction is skipped (mathematically identity).
- Between layers: AllGather of [SHARDR, 16] f32 shards; final mean-pool via
  ones-matmul + AllReduce.
"""
import sys

for _p in ("/opt/trn_rl_repo", "/root/.axon_site/_ro/trn_rl_repo"):
    if _p not in sys.path:
        sys.path.append(_p)

import numpy as np

import concourse.bacc as bacc
import concourse.tile as tile
import concourse.mybir as mybir
from concourse.bass_utils import run_bass_kernel_spmd

F32 = mybir.dt.float32
BF16 = mybir.dt.bfloat16
I16 = mybir.dt.int16
AF = mybir.ActivationFunctionType
OP = mybir.AluOpType

NCORES = 8
K = 8            # slots per row
MAXREG = 8
N = 25000
F_IN = 128
HID = 16
OUT = 32
GELEM = 64       # gcn table row, f32 words
AELEM = 128      # gat table row, f32 words
CHUNK_ROWS = 512


# ---------------------------------------------------------------- host prep
def _preprocess(edge_index):
    src = np.asarray(edge_index[0], np.int64)
    dst = np.asarray(edge_index[1], np.int64)
    loop = np.arange(N, dtype=np.int64)
    src = np.concatenate([src, loop])
    dst = np.concatenate([dst, loop])
    deg = np.bincount(dst, minlength=N)
    dinv = (1.0 / np.sqrt(deg.astype(np.float64))).astype(np.float32)

    core_of = (np.arange(N) % NCORES).astype(np.int64)
    rows_needed = -(-deg // K)
    assert rows_needed.max() <= MAXREG

    reg_nodes = [[None] * (MAXREG + 1) for _ in range(NCORES)]
    for c in range(NCORES):
        mine = np.where(core_of == c)[0]
        for r in range(1, MAXREG + 1):
            sel = mine[rows_needed[mine] == r]
            reg_nodes[c][r] = sel[np.argsort(-deg[sel], kind="stable")]

    reg_size = np.zeros(MAXREG + 1, np.int64)
    for r in range(1, MAXREG + 1):
        m = max(len(reg_nodes[c][r]) for c in range(NCORES))
        reg_size[r] = -(-m // 128) * 128 if m > 0 else 0
    SHARDR = int(reg_size[1:].sum())
    RTOT = int(sum(r * reg_size[r] for r in range(1, MAXREG + 1)))
    RPAD = -(-RTOT // CHUNK_ROWS) * CHUNK_ROWS
    NSLOT = K * RPAD
    TABN = NCORES * SHARDR
    PADROW = TABN
    assert PADROW < 32768, PADROW

    rank_base = np.cumsum([0] + [int(reg_size[r]) for r in range(1, MAXREG + 1)])
    row_base = np.cumsum([0] + [int(r * reg_size[r]) for r in range(1, MAXREG + 1)])

    rank_of = np.full(N, -1, np.int64)
    for c in range(NCORES):
        for ri, r in enumerate(range(1, MAXREG + 1)):
            nodes = reg_nodes[c][r]
            rank_of[nodes] = rank_base[ri] + np.arange(len(nodes))
    tabpos = core_of * SHARDR + rank_of

    cores = []
    for c in range(NCORES):
        slot_idx = np.full(NSLOT, PADROW, np.int64)
        mask = np.zeros(SHARDR, np.float32)
        nodes_local = np.full(SHARDR, -1, np.int64)
        for ri, r in enumerate(range(1, MAXREG + 1)):
            nodes = reg_nodes[c][r]
            mask[rank_base[ri]:rank_base[ri] + len(nodes)] = 1.0
            nodes_local[rank_base[ri]:rank_base[ri] + len(nodes)] = nodes

        emask = core_of[dst] == c
        es, ed = src[emask], dst[emask]
        order = np.argsort(ed, kind="stable")
        es, ed = es[order], ed[order]
        uniq, start_idx = np.unique(ed, return_index=True)
        pos = np.arange(len(ed)) - start_idx[np.searchsorted(uniq, ed)]
        r_of = rows_needed[ed]
        ri_of = r_of - 1
        j = pos // K
        k = pos % K
        rank_in_reg = rank_of[ed] - rank_base[ri_of]
        row = row_base[ri_of] + j * reg_size[r_of] + rank_in_reg
        slot_idx[k * RPAD + row] = tabpos[es]
        cores.append(dict(slot_idx=slot_idx, mask=mask, nodes_local=nodes_local))

    meta = dict(SHARDR=SHARDR, RPAD=RPAD, NSLOT=NSLOT, TABN=TABN, PADROW=PADROW,
                reg_size=reg_size, rank_base=rank_base, row_base=row_base,
                dinv=dinv, tabpos=tabpos)
    return cores, meta


def _wrap_idxs(idx):
    n = len(idx)
    w = idx.reshape(n // 16, 16).T.astype(np.int16)
    return np.tile(w, (8, 1))


def _to_pb(a, S):
    C = a.shape[1] if a.ndim > 1 else 1
    return a.reshape(S // 128, 128, C).transpose(1, 0, 2).reshape(128, -1)


# ---------------------------------------------------------------- kernel build
def _build(meta):
    S, RPAD, NSLOT, TABN = meta["SHARDR"], meta["RPAD"], meta["NSLOT"], meta["TABN"]
    SB = S // 128
    RB = RPAD // 128
    CRB = CHUNK_ROWS // 128
    NCH = RPAD // CHUNK_ROWS
    reg_size, rank_base, row_base = meta["reg_size"], meta["rank_base"], meta["row_base"]

    nc = bacc.Bacc("TRN2", target_bir_lowering=False, debug=False, num_swdge_queues=4)

    xs_in = nc.declare_dram_parameter("xs", [S, F_IN], F32, isOutput=False)
    dinv_in = nc.declare_dram_parameter("dinv", [128, SB], F32, isOutput=False)
    mask_in = nc.declare_dram_parameter("mask", [128, SB], F32, isOutput=False)
    idxs_in = nc.declare_dram_parameter("idxs", [128, NSLOT // 16], I16, isOutput=False)
    w1_in = nc.declare_dram_parameter("w1", [F_IN, HID], F32, isOutput=False)
    wga_in = nc.declare_dram_parameter("wga", [HID, 120], F32, isOutput=False)
    wgb_in = nc.declare_dram_parameter("wgb", [HID, 96], F32, isOutput=False)
    w2a_in = nc.declare_dram_parameter("w2a", [96, HID], F32, isOutput=False)
    w2b_in = nc.declare_dram_parameter("w2b", [96, HID], F32, isOutput=False)
    w3_in = nc.declare_dram_parameter("w3", [HID, HID], F32, isOutput=False)
    wlin_in = nc.declare_dram_parameter("wlin", [HID, OUT], F32, isOutput=False)
    bias_in = nc.declare_dram_parameter("biases", [1, 256], F32, isOutput=False)
    bg_in = nc.declare_dram_parameter("bg", [1, 192], F32, isOutput=False)
    padrow_in = nc.declare_dram_parameter("padrow", [1, AELEM], F32, isOutput=False)
    ident_in = nc.declare_dram_parameter("ident", [128, 128], F32, isOutput=False)
    out_ext = nc.declare_dram_parameter("out", [1, OUT], F32, isOutput=True)

    tab_gcn = nc.dram_tensor("tab_gcn", [TABN + 8, GELEM], F32)
    tab_gat = nc.dram_tensor("tab_gat", [TABN + 8, AELEM], F32)
    agin = nc.dram_tensor("agin", [S, HID], F32)
    agout = nc.dram_tensor("agout", [TABN, HID], F32, addr_space="Shared")
    aginT = nc.dram_tensor("aginT", [32, S], F32)
    agoutT = nc.dram_tensor("agoutT", [NCORES * 32, S], F32, addr_space="Shared")
    arin = nc.dram_tensor("arin", [HID, 1], F32)
    arout = nc.dram_tensor("arout", [HID, 1], F32, addr_space="Shared")

    cc_sem = nc.alloc_semaphore("ccs")
    io_sem = nc.alloc_semaphore("ios")
    st = {"cc": 0, "io": 0}

    with tile.TileContext(nc) as tc:
        with (
            tc.tile_pool(name="persist", bufs=1) as pp,
            tc.tile_pool(name="work", bufs=1) as wp,
            tc.tile_pool(name="gb", bufs=2) as gp,
            tc.tile_pool(name="ps", bufs=2, space="PSUM") as psp,
        ):
            idxs = pp.tile([128, NSLOT // 16], I16)
            nc.sync.dma_start(idxs[:], idxs_in[:])
            dinv = pp.tile([128, SB], F32)
            nc.sync.dma_start(dinv[:], dinv_in[:])
            mask = pp.tile([128, SB], F32)
            nc.sync.dma_start(mask[:], mask_in[:])
            w1 = pp.tile([F_IN, HID], F32)
            nc.sync.dma_start(w1[:], w1_in[:])
            wga = pp.tile([HID, 120], F32)
            nc.sync.dma_start(wga[:], wga_in[:])
            wgb = pp.tile([HID, 96], F32)
            nc.sync.dma_start(wgb[:], wgb_in[:])
            w2a = pp.tile([96, HID], F32)
            nc.sync.dma_start(w2a[:], w2a_in[:])
            w2b = pp.tile([96, HID], F32)
            nc.sync.dma_start(w2b[:], w2b_in[:])
            w3 = pp.tile([HID, HID], F32)
            nc.sync.dma_start(w3[:], w3_in[:])
            wlin = pp.tile([HID, OUT], F32)
            nc.sync.dma_start(wlin[:], wlin_in[:])
            ident = pp.tile([128, 128], F32)
            nc.sync.dma_start(ident[:], ident_in[:])
            bgt = pp.tile([128, 192], F32)
            nc.sync.dma_start(bgt[:], bg_in[:].partition_broadcast(128).squeeze(1))
            biasb = pp.tile([128, 256], F32)
            nc.sync.dma_start(biasb[:], bias_in[:].partition_broadcast(128).squeeze(1))

            nc.sync.dma_start(tab_gat[TABN:TABN + 1, :], padrow_in[:])
            zrow = pp.tile([1, GELEM], F32)
            nc.vector.memset(zrow[:], 0.0)
            nc.sync.dma_start(tab_gcn[TABN:TABN + 1, :], zrow[:])

            x1n = pp.tile([128, SB * HID], F32)
            x3n = pp.tile([128, SB * HID], F32)
            x4n = pp.tile([128, SB * HID], F32)
            gcnrows = pp.tile([128, RB * HID], F32)
            gatrows = pp.tile([128, RB * 192], BF16)
            denrows = pp.tile([128, RB * 12], F32)
            adrows = pp.tile([128, RB * 12], F32)

            def barrier():
                tc.strict_bb_all_engine_barrier()

            def collective(kind, op, src_dram, dst_dram):
                with tc.tile_critical():
                    nc.gpsimd.collective_compute(
                        kind, op, replica_groups=[list(range(NCORES))],
                        ins=[src_dram[:]], outs=[dst_dram[:]],
                    ).then_inc(cc_sem)
                    st["cc"] += 1
                    nc.gpsimd.wait_ge(cc_sem, st["cc"])
                barrier()

            def v3(t, c):
                return t[:].rearrange("p (b c) -> p b c", c=c)

            # per-tile node-major projection: dst[:, b, :] = src[:, b, :] @ Ws
            def project(dst3d, src3d, wlist, cdim):
                """wlist = [(W_ap, src_off, fdim), ...] summed over chunks."""
                for b in range(SB):
                    mm = psp.tile([cdim, 128], F32, tag="psB")
                    for i, (W, off, fd) in enumerate(wlist):
                        tp = psp.tile([fd, 128], F32, tag="psA")
                        nc.tensor.transpose(
                            tp[:], src3d[:, b, off:off + fd], ident[:])
                        tps = wp.tile([128, 128], F32, tag="ptps")
                        nc.scalar.copy(tps[0:fd, :], tp[:])
                        nc.tensor.matmul(mm[:], W, tps[0:fd, :],
                                         start=(i == 0), stop=(i == len(wlist) - 1))
                    mms = wp.tile([cdim, 128], F32, tag="pmms")
                    nc.scalar.copy(mms[:], mm[:])
                    tb = psp.tile([128, cdim], F32, tag="psC")
                    nc.tensor.transpose(tb[:], mms[:], ident[0:cdim, 0:cdim])
                    nc.scalar.copy(dst3d[:, b, :], tb[:])

            def ag_and_table(hn3d):
                nc.sync.dma_start(
                    agin[:].rearrange("(b p) f -> p b f", p=128), hn3d)
                barrier()
                collective("AllGather", OP.bypass, agin, agout)
                nc.sync.dma_start(tab_gcn[0:TABN, 0:HID], agout[:])
                barrier()

            def gcn_gather_pass():
                rows16 = v3(gcnrows, HID)
                for ch in range(NCH):
                    gs = [gp.tile([128, CRB * GELEM], F32, tag=f"gbuf{k}",
                                  name=f"gbuf{k}_{ch}") for k in range(K)]
                    g4s = [t[:].rearrange("p (b e) -> p b e", e=GELEM) for t in gs]
                    for k in range(K):
                        base = k * RPAD + ch * CHUNK_ROWS
                        nc.gpsimd.dma_gather(
                            g4s[k], tab_gcn[:],
                            idxs[:, base // 16:(base + CHUNK_ROWS) // 16],
                            num_idxs=CHUNK_ROWS, num_idxs_reg=CHUNK_ROWS,
                            elem_size=GELEM, queue_num=k % 4)
                    t4 = gp.tile([128, 4 * CRB * HID], F32, tag="gcn4")
                    t4v = t4[:].rearrange("p (k b c) -> p k b c", k=4, c=HID)
                    for k in range(4):
                        nc.vector.tensor_tensor(
                            t4v[:, k], g4s[k][:, :, 0:HID], g4s[k + 4][:, :, 0:HID],
                            op=OP.add)
                    t2 = gp.tile([128, 2 * CRB * HID], F32, tag="gcn2")
                    t2v = t2[:].rearrange("p (k b c) -> p k b c", k=2, c=HID)
                    nc.vector.tensor_tensor(t2v, t4v[:, 0:2], t4v[:, 2:4], op=OP.add)
                    nc.vector.tensor_tensor(
                        rows16[:, ch * CRB:(ch + 1) * CRB, :].unsqueeze(1),
                        t2v[:, 0:1], t2v[:, 1:2], op=OP.add)
                barrier()

            def combine_rows(rows3d, out3d):
                for ri, r in enumerate(range(1, MAXREG + 1)):
                    sz = int(reg_size[r])
                    if sz == 0:
                        continue
                    rb0 = int(row_base[ri]) // 128
                    kb0 = int(rank_base[ri]) // 128
                    nblk = sz // 128
                    dst = out3d[:, kb0:kb0 + nblk, :]
                    if r == 1:
                        nc.vector.tensor_copy(dst, rows3d[:, rb0:rb0 + nblk, :])
                    else:
                        nc.vector.tensor_tensor(
                            dst, rows3d[:, rb0:rb0 + nblk, :],
                            rows3d[:, rb0 + nblk:rb0 + 2 * nblk, :], op=OP.add)
                        for j in range(2, r):
                            nc.vector.tensor_tensor(
                                dst, dst,
                                rows3d[:, rb0 + j * nblk:rb0 + (j + 1) * nblk, :],
                                op=OP.add)

            def gcn_epilogue(xdst3d, bias_off, residual=None):
                agg = wp.tile([128, SB * HID], F32, tag="agg")
                agg3 = v3(agg, HID)
                combine_rows(v3(gcnrows, HID), agg3)
                nc.vector.tensor_tensor(
                    agg3, agg3, dinv[:].unsqueeze(2).broadcast_to([128, SB, HID]),
                    op=OP.mult)
                nc.vector.tensor_tensor(
                    agg3, agg3,
                    biasb[:, bias_off:bias_off + HID].unsqueeze(1).broadcast_to(
                        [128, SB, HID]), op=OP.add)
                nc.scalar.activation(xdst3d, agg3, AF.Relu)
                if residual is not None:
                    nc.vector.tensor_tensor(xdst3d, xdst3d, residual, op=OP.add)
                barrier()

            # ================= GCN1 =================
            xsc = wp.tile([128, SB * F_IN], F32, tag="xsc")
            nc.sync.dma_start(
                v3(xsc, F_IN), xs_in[:].rearrange("(b p) f -> p b f", p=128))
            nc.vector.tensor_tensor(
                v3(xsc, F_IN), v3(xsc, F_IN),
                dinv[:].unsqueeze(2).broadcast_to([128, SB, F_IN]), op=OP.mult)
            h1n = wp.tile([128, SB * HID], F32, tag="hn")
            project(v3(h1n, HID), v3(xsc, F_IN), [(w1[:], 0, F_IN)], HID)
            barrier()
            ag_and_table(v3(h1n, HID))
            gcn_gather_pass()
            gcn_epilogue(v3(x1n, HID), 0)

            # ================= GAT prep =================
            # per-tile: x1T tile -> aginT cols; adT tile -> adr
            adr = wp.tile([128, SB * 12], F32, tag="adr")
            zc = wp.tile([16, 128], F32, tag="zc")
            nc.vector.memset(zc[:], 0.0)
            for b in range(SB):
                tp = psp.tile([HID, 128], F32, tag="psA")
                nc.tensor.transpose(tp[:], v3(x1n, HID)[:, b, :], ident[:])
                tps = wp.tile([HID, 128], F32, tag="x1tt")
                nc.scalar.copy(tps[:], tp[:])
                nc.sync.dma_start(aginT[0:16, b * 128:(b + 1) * 128], tps[:])
                nc.sync.dma_start(aginT[16:32, b * 128:(b + 1) * 128], zc[:])
                ad_ps = psp.tile([12, 128], F32, tag="psB")
                nc.tensor.matmul(ad_ps[:], wga[:, 108:120], tps[:],
                                 start=True, stop=True)
                ad_sb = wp.tile([12, 128], F32, tag="adsb")
                nc.scalar.copy(ad_sb[:], ad_ps[:])
                tb = psp.tile([128, 12], F32, tag="psC")
                nc.tensor.transpose(tb[:], ad_sb[:], ident[0:12, 0:12])
                nc.scalar.copy(v3(adr, 12)[:, b, :], tb[:])
            barrier()
            adrows3 = v3(adrows, 12)
            adr3 = v3(adr, 12)
            for ri, r in enumerate(range(1, MAXREG + 1)):
                sz = int(reg_size[r])
                if sz == 0:
                    continue
                rb0, kb0, nblk = int(row_base[ri]) // 128, int(rank_base[ri]) // 128, sz // 128
                for j in range(r):
                    nc.vector.tensor_copy(
                        adrows3[:, rb0 + j * nblk:rb0 + (j + 1) * nblk, :],
                        adr3[:, kb0:kb0 + nblk, :])
            barrier()
            collective("AllGather", OP.bypass, aginT, agoutT)
            # build tab_gat per shard in column-chunks of <=1024 nodes
            for s0 in range(NCORES):
                loc = 0
                while loc < S:
                    cw = min(1024, S - loc)
                    gtiles = cw // 128
                    strip0 = wp.tile([96, 1024], BF16, tag="strip0")
                    strip1 = wp.tile([96, 1024], BF16, tag="strip1")
                    asadT = wp.tile([24, 1024], F32, tag="asadT")
                    nsub = -(-cw // 512)
                    for ci in range(nsub):
                        sc0 = ci * 512
                        scw = min(512, cw - sc0)
                        xr = wp.tile([16, 512], F32, tag="xr")
                        nc.sync.dma_start(
                            xr[:, 0:scw],
                            agoutT[32 * s0:32 * s0 + 16, loc + sc0:loc + sc0 + scw])
                        pA = psp.tile([120, 512], F32, tag="psA")
                        nc.tensor.matmul(pA[:, 0:scw], wga[:], xr[:, 0:scw],
                                         start=True, stop=True)
                        nc.scalar.copy(strip0[:, sc0:sc0 + scw], pA[0:96, 0:scw])
                        nc.scalar.copy(asadT[:, sc0:sc0 + scw], pA[96:120, 0:scw])
                        pB = psp.tile([96, 512], F32, tag="psB")
                        nc.tensor.matmul(pB[:, 0:scw], wgb[:], xr[:, 0:scw],
                                         start=True, stop=True)
                        nc.scalar.copy(strip1[:, sc0:sc0 + scw], pB[:, 0:scw])
                    rowblk = wp.tile([128, 8 * AELEM], F32, tag="rowblk")
                    rb3 = rowblk[:].rearrange("p (g e) -> p g e", e=AELEM)
                    nc.vector.memset(rowblk[:], 0.0)
                    hh0 = rb3[:, 0:gtiles, 16:64].bitcast(BF16)
                    nc.sync.dma_start(hh0, strip0[:, 0:gtiles * 128], transpose=True)
                    hh1 = rb3[:, 0:gtiles, 64:112].bitcast(BF16)
                    nc.sync.dma_start(hh1, strip1[:, 0:gtiles * 128], transpose=True)
                    for g in range(gtiles):
                        pt = psp.tile([128, 24], F32, tag="psC")
                        nc.tensor.transpose(
                            pt[:], asadT[:, g * 128:(g + 1) * 128],
                            ident[0:24, 0:24])
                        nc.scalar.copy(rb3[:, g, 0:12], pt[:, 0:12])
                    c0 = s0 * S + loc
                    nc.sync.dma_start(
                        tab_gat[c0:c0 + cw, :].rearrange("(g p) e -> p g e", p=128),
                        rb3[:, 0:gtiles, :])
                    loc += cw
            barrier()

            # ================= GAT gather =================
            grv = gatrows[:].rearrange("p (b c) -> p b c", c=192)
            drv = v3(denrows, 12)
            for ch in range(NCH):
                gs = [gp.tile([128, CRB * AELEM], F32, tag=f"agbuf{k}",
                              name=f"agbuf{k}_{ch}") for k in range(K)]
                g4s = [t[:].rearrange("p (b e) -> p b e", e=AELEM) for t in gs]
                for k in range(K):
                    base = k * RPAD + ch * CHUNK_ROWS
                    nc.gpsimd.dma_gather(
                        g4s[k], tab_gat[:],
                        idxs[:, base // 16:(base + CHUNK_ROWS) // 16],
                        num_idxs=CHUNK_ROWS, num_idxs_reg=CHUNK_ROWS,
                        elem_size=AELEM, queue_num=k % 4)
                ex = gp.tile([128, K * CRB * 12], F32, tag="ex")
                exv = ex[:].rearrange("p (k b h) -> p k b h", k=K, h=12)
                for k in range(K):
                    nc.vector.tensor_tensor(
                        exv[:, k], g4s[k][:, :, 0:12],
                        adrows3[:, ch * CRB:(ch + 1) * CRB, :], op=OP.add)
                lk = gp.tile([128, K * CRB * 12], F32, tag="lk")
                lkv = lk[:].rearrange("p (k b h) -> p k b h", k=K, h=12)
                nc.scalar.mul(lkv, exv, 0.2)
                nc.vector.tensor_tensor(exv, exv, lkv, op=OP.max)
                nc.scalar.activation(exv, exv, AF.Exp)
                nc.vector.tensor_tensor(lkv[:, 0:4], exv[:, 0:4], exv[:, 4:8], op=OP.add)
                nc.vector.tensor_tensor(lkv[:, 0:2], lkv[:, 0:2], lkv[:, 2:4], op=OP.add)
                nc.vector.tensor_tensor(
                    drv[:, ch * CRB:(ch + 1) * CRB, :].unsqueeze(1),
                    lkv[:, 0:1], lkv[:, 1:2], op=OP.add)
                exb = gp.tile([128, K * CRB * 12], BF16, tag="exb")
                exbv = exb[:].rearrange("p (k b h) -> p k b h", k=K, h=12)
                nc.vector.tensor_copy(exbv, exv)
                hhs = [g4s[k][:, :, 16:112].bitcast(BF16).rearrange(
                    "p b (h c) -> p b h c", c=HID) for k in range(K)]
                for k in range(K):
                    nc.vector.tensor_tensor(
                        hhs[k], hhs[k],
                        exbv[:, k].unsqueeze(3).broadcast_to([128, CRB, 12, HID]),
                        op=OP.mult)
                hhf = [h.rearrange("p b h c -> p b (h c)") for h in hhs]
                for k in range(4):
                    nc.vector.tensor_tensor(hhf[k], hhf[k], hhf[k + 4], op=OP.add)
                for k in range(2):
                    nc.vector.tensor_tensor(hhf[k], hhf[k], hhf[k + 2], op=OP.add)
                nc.vector.tensor_tensor(
                    grv[:, ch * CRB:(ch + 1) * CRB, :], hhf[0], hhf[1], op=OP.add)
            barrier()

            # ================= GAT epilogue =================
            num = wp.tile([128, SB * 192], F32, tag="num")
            num3 = v3(num, 192)
            combine_rows(grv, num3)
            den = wp.tile([128, SB * 12], F32, tag="den")
            den3 = v3(den, 12)
            combine_rows(drv, den3)
            nc.vector.reciprocal(den3, den3)
            num4 = num3.rearrange("p b (h c) -> p b h c", c=HID)
            nc.vector.tensor_tensor(
                num4, num4,
                den3.unsqueeze(3).broadcast_to([128, SB, 12, HID]), op=OP.mult)
            nc.vector.tensor_tensor(
                num3, num3, bgt[:].unsqueeze(1).broadcast_to([128, SB, 192]),
                op=OP.add)
            el1 = wp.tile([128, SB * 192], F32, tag="el1")
            el13 = v3(el1, 192)
            nc.vector.tensor_scalar_min(el13, num3, 0.0)
            nc.scalar.activation(el13, el13, AF.Exp)
            nc.scalar.activation(num3, num3, AF.Relu)
            nc.vector.tensor_tensor(num3, num3, el13, op=OP.add)
            nc.vector.tensor_scalar_add(num3, num3, -1.0)
            nc.vector.tensor_tensor(
                num3, num3, dinv[:].unsqueeze(2).broadcast_to([128, SB, 192]),
                op=OP.mult)
            barrier()

            # ================= GCN2 =================
            h2n = wp.tile([128, SB * HID], F32, tag="hn")
            project(v3(h2n, HID), num3,
                    [(w2a[:], 0, 96), (w2b[:], 96, 96)], HID)
            barrier()
            ag_and_table(v3(h2n, HID))
            gcn_gather_pass()
            gcn_epilogue(v3(x3n, HID), 16)

            # ================= GCN3 =================
            x3sc = wp.tile([128, SB * HID], F32, tag="x3sc")
            nc.vector.tensor_tensor(
                v3(x3sc, HID), v3(x3n, HID),
                dinv[:].unsqueeze(2).broadcast_to([128, SB, HID]), op=OP.mult)
            h3n = wp.tile([128, SB * HID], F32, tag="hn")
            project(v3(h3n, HID), v3(x3sc, HID), [(w3[:], 0, HID)], HID)
            barrier()
            ag_and_table(v3(h3n, HID))
            gcn_gather_pass()
            gcn_epilogue(v3(x4n, HID), 32, residual=v3(x3n, HID))

            # ================= mean pool + linear =================
            x4m = wp.tile([128, SB * HID], F32, tag="x4m")
            nc.vector.tensor_tensor(
                v3(x4m, HID), v3(x4n, HID),
                mask[:].unsqueeze(2).broadcast_to([128, SB, HID]), op=OP.mult)
            ones = pp.tile([128, 1], F32)
            nc.vector.memset(ones[:], 1.0)
            pool_ps = psp.tile([HID, 1], F32, tag="psB")
            for b in range(SB):
                nc.tensor.matmul(
                    pool_ps[:], v3(x4m, HID)[:, b, :], ones[:],
                    start=(b == 0), stop=(b == SB - 1))
            pool = wp.tile([HID, 1], F32, tag="pool_sb")
            nc.scalar.copy(pool[:], pool_ps[:])
            barrier()
            with tc.tile_critical():
                nc.gpsimd.dma_start(arin[:], pool[:]).then_inc(io_sem, 16)
                st["io"] += 16
                nc.gpsimd.wait_ge(io_sem, st["io"])
                nc.gpsimd.collective_compute(
                    "AllReduce", OP.add, replica_groups=[list(range(NCORES))],
                    ins=[arin[:]], outs=[arout[:]],
                ).then_inc(cc_sem)
                st["cc"] += 1
                nc.gpsimd.wait_ge(cc_sem, st["cc"])
            barrier()
            poolg = wp.tile([HID, 1], F32, tag="poolg")
            nc.sync.dma_start(poolg[:], arout[:])
            nc.scalar.mul(poolg[:], poolg[:], 1.0 / N)
            out_ps = psp.tile([1, OUT], F32, tag="psB")
            nc.tensor.matmul(out_ps[:], poolg[:], wlin[:], start=True, stop=True)
            outt = wp.tile([1, OUT], F32, tag="outt")
            nc.scalar.copy(outt[:], out_ps[:])
            nc.vector.tensor_tensor(outt[:], outt[:], biasb[0:1, 48:48 + OUT],
                                    op=OP.add)
            nc.sync.dma_start(out_ext[:], outt[:])

    nc.compile()
    return nc


# ---------------------------------------------------------------- entry point
def _make_in_maps(inputs, cores, meta):
    x = np.asarray(inputs["x"], np.float32)
    S = meta["SHARDR"]
    W1 = np.asarray(inputs["W1"], np.float32)
    Wg = np.asarray(inputs["Wg"], np.float32)
    att_src = np.asarray(inputs["att_src"], np.float32)
    att_dst = np.asarray(inputs["att_dst"], np.float32)
    W2 = np.asarray(inputs["W2"], np.float32)
    W3 = np.asarray(inputs["W3"], np.float32)
    Wlin = np.asarray(inputs["Wlin"], np.float32)

    Wg3 = Wg.reshape(16, 12, 16)
    Was = np.einsum("khc,hc->kh", Wg3, att_src).astype(np.float32)
    Wad = np.einsum("khc,hc->kh", Wg3, att_dst).astype(np.float32)
    wga = np.concatenate([Wg[:, 0:96], Was, Wad], axis=1)  # [16, 120]
    wgb = np.ascontiguousarray(Wg[:, 96:192])

    biases = np.zeros((1, 256), np.float32)
    biases[0, 0:16] = np.asarray(inputs["b1"], np.float32)
    biases[0, 16:32] = np.asarray(inputs["b2"], np.float32)
    biases[0, 32:48] = np.asarray(inputs["b3"], np.float32)
    biases[0, 48:80] = np.asarray(inputs["blin"], np.float32)
    bg_row = np.asarray(inputs["bg"], np.float32)[None, :]
    padrow = np.zeros((1, AELEM), np.float32)
    padrow[0, 0:12] = -200.0
    ident = np.eye(128, dtype=np.float32)

    xsq = x.reshape(N, F_IN)
    dinv = meta["dinv"]
    in_maps = []
    for c in range(NCORES):
        nl = cores[c]["nodes_local"]
        real = nl >= 0
        xs = np.zeros((S, F_IN), np.float32)
        xs[real] = xsq[nl[real]]
        dv = np.zeros(S, np.float32)
        dv[real] = dinv[nl[real]]
        in_maps.append({
            "xs": xs,
            "dinv": np.ascontiguousarray(_to_pb(dv[:, None], S), dtype=np.float32),
            "mask": np.ascontiguousarray(_to_pb(cores[c]["mask"][:, None], S), dtype=np.float32),
            "idxs": _wrap_idxs(cores[c]["slot_idx"]),
            "w1": W1, "wga": wga, "wgb": wgb,
            "w2a": np.ascontiguousarray(W2[0:96]),
            "w2b": np.ascontiguousarray(W2[96:192]),
            "w3": W3, "wlin": Wlin,
            "biases": biases, "bg": bg_row, "padrow": padrow, "ident": ident,
        })
    return in_maps


def run(inputs, trace=False):
    edge_index = np.asarray(inputs["edge_index"])
    cores, meta = _preprocess(edge_index)
    in_maps = _make_in_maps(inputs, cores, meta)
    nc = _build(meta)
    res = run_bass_kernel_spmd(nc, in_maps, list(range(NCORES)), trace=trace)
    return res


def kernel(**inputs):
    return run(inputs).results[0]["out"]


if __name__ == "__main__":
    import reference
    inputs = reference.setup_inputs()
    inputs = {k: np.asarray(v) for k, v in inputs.items()}
    got = kernel(**inputs)
    exp = np.asarray(reference.reference(**inputs))
    rel = np.abs(got - exp).max() / np.abs(exp).max()
    print("rel err:", rel)


# revision 18
# speedup vs baseline: 1.1347x; 1.1347x over previous
"""Trainium2 Bass kernel for nn_EnhancedGCN (GCN -> GAT -> GCN -> GCN -> mean -> linear).

Strategy (8 NeuronCores, dst-sharded message passing):
- Host: add self loops, in-degree -> dinv (compile-time), relabel nodes
  (core = n % 8, rank grouped by region r = ceil(indeg/K), K=8 slots/row,
  region sizes padded uniform across cores). Slot i = k*RPAD + row holds the
  table index of the k-th in-edge source of `row`; gathered slot i lands at
  SBUF [i%128, i//128], i.e. rows are partition-wrapped and k is a free-dim
  plane -> K-reduce is log-step DVE adds, region combine is block-offset adds.
- Aggregation passes gather from per-core HBM tables via SWDGE dma_gather
  (chunks of 1024 idxs, 4 queues). GCN tables: [TABN+8, 64] f32 rows (16
  used). GAT table: [TABN+8, 128] f32-word rows = [a_src 12 f32 | pad |
  hh 192 bf16 | pad]; the pad row has a_src=-200 so exp() ~ 0.
- a_src/a_dst attention vectors are folded into Wg on the host (Was/Wad);
  softmax max-subtraction is skipped (mathematically identity).
- Between layers: AllGather of [SHARDR, 16] f32 shards; final mean-pool via
  ones-matmul + AllReduce.
"""
import sys

for _p in ("/opt/trn_rl_repo", "/root/.axon_site/_ro/trn_rl_repo"):
    if _p not in sys.path:
        sys.path.append(_p)

import numpy as np

import concourse.bacc as bacc
import concourse.tile as tile
import concourse.mybir as mybir
from concourse.bass_utils import run_bass_kernel_spmd

F32 = mybir.dt.float32
BF16 = mybir.dt.bfloat16
I16 = mybir.dt.int16
AF = mybir.ActivationFunctionType
OP = mybir.AluOpType

NCORES = 8
K = 4            # slots per row
MAXREG = 16
N = 25000
F_IN = 128
HID = 16
OUT = 32
GELEM = 64       # gcn table row, f32 words
AELEM = 128      # gat table row, f32 words
CHUNK_ROWS = 512


# ---------------------------------------------------------------- host prep
def _preprocess(edge_index):
    src = np.asarray(edge_index[0], np.int64)
    dst = np.asarray(edge_index[1], np.int64)
    loop = np.arange(N, dtype=np.int64)
    src = np.concatenate([src, loop])
    dst = np.concatenate([dst, loop])
    deg = np.bincount(dst, minlength=N)
    dinv = (1.0 / np.sqrt(deg.astype(np.float64))).astype(np.float32)

    core_of = (np.arange(N) % NCORES).astype(np.int64)
    rows_needed = -(-deg // K)
    assert rows_needed.max() <= MAXREG

    reg_nodes = [[None] * (MAXREG + 1) for _ in range(NCORES)]
    for c in range(NCORES):
        mine = np.where(core_of == c)[0]
        for r in range(1, MAXREG + 1):
            sel = mine[rows_needed[mine] == r]
            reg_nodes[c][r] = sel[np.argsort(-deg[sel], kind="stable")]

    reg_size = np.zeros(MAXREG + 1, np.int64)
    for r in range(1, MAXREG + 1):
        m = max(len(reg_nodes[c][r]) for c in range(NCORES))
        reg_size[r] = -(-m // 128) * 128 if m > 0 else 0
    SHARDR = int(reg_size[1:].sum())
    RTOT = int(sum(r * reg_size[r] for r in range(1, MAXREG + 1)))
    RPAD = -(-RTOT // CHUNK_ROWS) * CHUNK_ROWS
    NSLOT = K * RPAD
    TABN = NCORES * SHARDR
    PADROW = TABN
    assert PADROW < 32768, PADROW

    rank_base = np.cumsum([0] + [int(reg_size[r]) for r in range(1, MAXREG + 1)])
    row_base = np.cumsum([0] + [int(r * reg_size[r]) for r in range(1, MAXREG + 1)])

    rank_of = np.full(N, -1, np.int64)
    for c in range(NCORES):
        for ri, r in enumerate(range(1, MAXREG + 1)):
            nodes = reg_nodes[c][r]
            rank_of[nodes] = rank_base[ri] + np.arange(len(nodes))
    tabpos = core_of * SHARDR + rank_of

    cores = []
    for c in range(NCORES):
        slot_idx = np.full(NSLOT, PADROW, np.int64)
        mask = np.zeros(SHARDR, np.float32)
        nodes_local = np.full(SHARDR, -1, np.int64)
        for ri, r in enumerate(range(1, MAXREG + 1)):
            nodes = reg_nodes[c][r]
            mask[rank_base[ri]:rank_base[ri] + len(nodes)] = 1.0
            nodes_local[rank_base[ri]:rank_base[ri] + len(nodes)] = nodes

        emask = core_of[dst] == c
        es, ed = src[emask], dst[emask]
        order = np.argsort(ed, kind="stable")
        es, ed = es[order], ed[order]
        uniq, start_idx = np.unique(ed, return_index=True)
        pos = np.arange(len(ed)) - start_idx[np.searchsorted(uniq, ed)]
        r_of = rows_needed[ed]
        ri_of = r_of - 1
        j = pos // K
        k = pos % K
        rank_in_reg = rank_of[ed] - rank_base[ri_of]
        row = row_base[ri_of] + j * reg_size[r_of] + rank_in_reg
        slot_idx[k * RPAD + row] = tabpos[es]
        cores.append(dict(slot_idx=slot_idx, mask=mask, nodes_local=nodes_local))

    meta = dict(SHARDR=SHARDR, RPAD=RPAD, NSLOT=NSLOT, TABN=TABN, PADROW=PADROW,
                reg_size=reg_size, rank_base=rank_base, row_base=row_base,
                dinv=dinv, tabpos=tabpos)
    return cores, meta


def _wrap_idxs(idx):
    n = len(idx)
    w = idx.reshape(n // 16, 16).T.astype(np.int16)
    return np.tile(w, (8, 1))


def _to_pb(a, S):
    C = a.shape[1] if a.ndim > 1 else 1
    return a.reshape(S // 128, 128, C).transpose(1, 0, 2).reshape(128, -1)


# ---------------------------------------------------------------- kernel build
def _build(meta):
    S, RPAD, NSLOT, TABN = meta["SHARDR"], meta["RPAD"], meta["NSLOT"], meta["TABN"]
    SB = S // 128
    RB = RPAD // 128
    CRB = CHUNK_ROWS // 128
    NCH = RPAD // CHUNK_ROWS
    reg_size, rank_base, row_base = meta["reg_size"], meta["rank_base"], meta["row_base"]

    nc = bacc.Bacc("TRN2", target_bir_lowering=False, debug=False, num_swdge_queues=4)

    xs_in = nc.declare_dram_parameter("xs", [S, F_IN], F32, isOutput=False)
    dinv_in = nc.declare_dram_parameter("dinv", [128, SB], F32, isOutput=False)
    mask_in = nc.declare_dram_parameter("mask", [128, SB], F32, isOutput=False)
    idxs_in = nc.declare_dram_parameter("idxs", [128, NSLOT // 16], I16, isOutput=False)
    w1_in = nc.declare_dram_parameter("w1", [F_IN, HID], F32, isOutput=False)
    wga_in = nc.declare_dram_parameter("wga", [HID, 120], F32, isOutput=False)
    wgb_in = nc.declare_dram_parameter("wgb", [HID, 96], F32, isOutput=False)
    w2a_in = nc.declare_dram_parameter("w2a", [96, HID], F32, isOutput=False)
    w2b_in = nc.declare_dram_parameter("w2b", [96, HID], F32, isOutput=False)
    w3_in = nc.declare_dram_parameter("w3", [HID, HID], F32, isOutput=False)
    wlin_in = nc.declare_dram_parameter("wlin", [HID, OUT], F32, isOutput=False)
    bias_in = nc.declare_dram_parameter("biases", [1, 256], F32, isOutput=False)
    bg_in = nc.declare_dram_parameter("bg", [1, 192], F32, isOutput=False)
    padrow_in = nc.declare_dram_parameter("padrow", [1, AELEM], F32, isOutput=False)
    ident_in = nc.declare_dram_parameter("ident", [128, 128], F32, isOutput=False)
    out_ext = nc.declare_dram_parameter("out", [1, OUT], F32, isOutput=True)

    tab_gcn = nc.dram_tensor("tab_gcn", [TABN + 8, GELEM], F32)
    tab_gat = nc.dram_tensor("tab_gat", [TABN + 8, AELEM], F32)
    agin = nc.dram_tensor("agin", [S, HID], F32)
    agout = nc.dram_tensor("agout", [TABN, HID], F32, addr_space="Shared")
    aginT = nc.dram_tensor("aginT", [32, S], F32)
    agoutT = nc.dram_tensor("agoutT", [NCORES * 32, S], F32, addr_space="Shared")
    arin = nc.dram_tensor("arin", [HID, 1], F32)
    arout = nc.dram_tensor("arout", [HID, 1], F32, addr_space="Shared")

    cc_sem = nc.alloc_semaphore("ccs")
    io_sem = nc.alloc_semaphore("ios")
    st = {"cc": 0, "io": 0}

    with tile.TileContext(nc) as tc:
        with (
            tc.tile_pool(name="persist", bufs=1) as pp,
            tc.tile_pool(name="work", bufs=1) as wp,
            tc.tile_pool(name="gb", bufs=2) as gp,
            tc.tile_pool(name="ps", bufs=2, space="PSUM") as psp,
        ):
            idxs = pp.tile([128, NSLOT // 16], I16)
            nc.sync.dma_start(idxs[:], idxs_in[:])
            dinv = pp.tile([128, SB], F32)
            nc.sync.dma_start(dinv[:], dinv_in[:])
            mask = pp.tile([128, SB], F32)
            nc.sync.dma_start(mask[:], mask_in[:])
            w1 = pp.tile([F_IN, HID], F32)
            nc.sync.dma_start(w1[:], w1_in[:])
            wga = pp.tile([HID, 120], F32)
            nc.sync.dma_start(wga[:], wga_in[:])
            wgb = pp.tile([HID, 96], F32)
            nc.sync.dma_start(wgb[:], wgb_in[:])
            w2a = pp.tile([96, HID], F32)
            nc.sync.dma_start(w2a[:], w2a_in[:])
            w2b = pp.tile([96, HID], F32)
            nc.sync.dma_start(w2b[:], w2b_in[:])
            w3 = pp.tile([HID, HID], F32)
            nc.sync.dma_start(w3[:], w3_in[:])
            wlin = pp.tile([HID, OUT], F32)
            nc.sync.dma_start(wlin[:], wlin_in[:])
            ident = pp.tile([128, 128], F32)
            nc.sync.dma_start(ident[:], ident_in[:])
            bgt = pp.tile([128, 192], F32)
            nc.sync.dma_start(bgt[:], bg_in[:].partition_broadcast(128).squeeze(1))
            biasb = pp.tile([128, 256], F32)
            nc.sync.dma_start(biasb[:], bias_in[:].partition_broadcast(128).squeeze(1))

            nc.sync.dma_start(tab_gat[TABN:TABN + 1, :], padrow_in[:])
            zrow = pp.tile([1, GELEM], F32)
            nc.vector.memset(zrow[:], 0.0)
            nc.sync.dma_start(tab_gcn[TABN:TABN + 1, :], zrow[:])

            x1n = pp.tile([128, SB * HID], F32)
            x3n = pp.tile([128, SB * HID], F32)
            x4n = pp.tile([128, SB * HID], F32)
            gcnrows = pp.tile([128, RB * HID], F32)
            gatrows = pp.tile([128, RB * 192], BF16)
            denrows = pp.tile([128, RB * 12], F32)
            adrows = pp.tile([128, RB * 12], F32)

            def barrier():
                tc.strict_bb_all_engine_barrier()

            def collective(kind, op, src_dram, dst_dram):
                with tc.tile_critical():
                    nc.gpsimd.collective_compute(
                        kind, op, replica_groups=[list(range(NCORES))],
                        ins=[src_dram[:]], outs=[dst_dram[:]],
                    ).then_inc(cc_sem)
                    st["cc"] += 1
                    nc.gpsimd.wait_ge(cc_sem, st["cc"])
                barrier()

            def v3(t, c):
                return t[:].rearrange("p (b c) -> p b c", c=c)

            # per-tile node-major projection: dst[:, b, :] = src[:, b, :] @ Ws
            def project(dst3d, src3d, wlist, cdim):
                """wlist = [(W_ap, src_off, fdim), ...] summed over chunks."""
                for b in range(SB):
                    mm = psp.tile([cdim, 128], F32, tag="psB")
                    for i, (W, off, fd) in enumerate(wlist):
                        tp = psp.tile([fd, 128], F32, tag="psA")
                        nc.tensor.transpose(
                            tp[:], src3d[:, b, off:off + fd], ident[:])
                        tps = wp.tile([128, 128], F32, tag="ptps")
                        nc.scalar.copy(tps[0:fd, :], tp[:])
                        nc.tensor.matmul(mm[:], W, tps[0:fd, :],
                                         start=(i == 0), stop=(i == len(wlist) - 1))
                    mms = wp.tile([cdim, 128], F32, tag="pmms")
                    nc.scalar.copy(mms[:], mm[:])
                    tb = psp.tile([128, cdim], F32, tag="psC")
                    nc.tensor.transpose(tb[:], mms[:], ident[0:cdim, 0:cdim])
                    nc.scalar.copy(dst3d[:, b, :], tb[:])

            def ag_and_table(hn3d):
                nc.sync.dma_start(
                    agin[:].rearrange("(b p) f -> p b f", p=128), hn3d)
                barrier()
                collective("AllGather", OP.bypass, agin, agout)
                nc.sync.dma_start(tab_gcn[0:TABN, 0:HID], agout[:])
                barrier()

            def gcn_gather_pass():
                rows16 = v3(gcnrows, HID)
                for ch in range(NCH):
                    gs = [gp.tile([128, CRB * GELEM], F32, tag=f"gbuf{k}",
                                  name=f"gbuf{k}_{ch}") for k in range(K)]
                    g4s = [t[:].rearrange("p (b e) -> p b e", e=GELEM) for t in gs]
                    for k in range(K):
                        base = k * RPAD + ch * CHUNK_ROWS
                        nc.gpsimd.dma_gather(
                            g4s[k], tab_gcn[:],
                            idxs[:, base // 16:(base + CHUNK_ROWS) // 16],
                            num_idxs=CHUNK_ROWS, num_idxs_reg=CHUNK_ROWS,
                            elem_size=GELEM, queue_num=k % 4)
                    t2 = gp.tile([128, 2 * CRB * HID], F32, tag="gcn2")
                    t2v = t2[:].rearrange("p (k b c) -> p k b c", k=2, c=HID)
                    for k in range(2):
                        nc.vector.tensor_tensor(
                            t2v[:, k], g4s[k][:, :, 0:HID], g4s[k + 2][:, :, 0:HID],
                            op=OP.add)
                    nc.vector.tensor_tensor(
                        rows16[:, ch * CRB:(ch + 1) * CRB, :].unsqueeze(1),
                        t2v[:, 0:1], t2v[:, 1:2], op=OP.add)
                barrier()

            def combine_rows(rows3d, out3d):
                for ri, r in enumerate(range(1, MAXREG + 1)):
                    sz = int(reg_size[r])
                    if sz == 0:
                        continue
                    rb0 = int(row_base[ri]) // 128
                    kb0 = int(rank_base[ri]) // 128
                    nblk = sz // 128
                    dst = out3d[:, kb0:kb0 + nblk, :]
                    if r == 1:
                        nc.vector.tensor_copy(dst, rows3d[:, rb0:rb0 + nblk, :])
                    else:
                        nc.vector.tensor_tensor(
                            dst, rows3d[:, rb0:rb0 + nblk, :],
                            rows3d[:, rb0 + nblk:rb0 + 2 * nblk, :], op=OP.add)
                        for j in range(2, r):
                            nc.vector.tensor_tensor(
                                dst, dst,
                                rows3d[:, rb0 + j * nblk:rb0 + (j + 1) * nblk, :],
                                op=OP.add)

            def gcn_epilogue(xdst3d, bias_off, residual=None):
                agg = wp.tile([128, SB * HID], F32, tag="agg")
                agg3 = v3(agg, HID)
                combine_rows(v3(gcnrows, HID), agg3)
                nc.vector.tensor_tensor(
                    agg3, agg3, dinv[:].unsqueeze(2).broadcast_to([128, SB, HID]),
                    op=OP.mult)
                nc.vector.tensor_tensor(
                    agg3, agg3,
                    biasb[:, bias_off:bias_off + HID].unsqueeze(1).broadcast_to(
                        [128, SB, HID]), op=OP.add)
                nc.scalar.activation(xdst3d, agg3, AF.Relu)
                if residual is not None:
                    nc.vector.tensor_tensor(xdst3d, xdst3d, residual, op=OP.add)
                barrier()

            # ================= GCN1 =================
            xsc = wp.tile([128, SB * F_IN], F32, tag="num")
            nc.sync.dma_start(
                v3(xsc, F_IN), xs_in[:].rearrange("(b p) f -> p b f", p=128))
            nc.vector.tensor_tensor(
                v3(xsc, F_IN), v3(xsc, F_IN),
                dinv[:].unsqueeze(2).broadcast_to([128, SB, F_IN]), op=OP.mult)
            h1n = wp.tile([128, SB * HID], F32, tag="hn")
            project(v3(h1n, HID), v3(xsc, F_IN), [(w1[:], 0, F_IN)], HID)
            barrier()
            ag_and_table(v3(h1n, HID))
            gcn_gather_pass()
            gcn_epilogue(v3(x1n, HID), 0)

            # ================= GAT prep =================
            # per-tile: x1T tile -> aginT cols; adT tile -> adr
            adr = wp.tile([128, SB * 12], F32, tag="adr")
            zc = wp.tile([16, 128], F32, tag="zc")
            nc.vector.memset(zc[:], 0.0)
            for b in range(SB):
                tp = psp.tile([HID, 128], F32, tag="psA")
                nc.tensor.transpose(tp[:], v3(x1n, HID)[:, b, :], ident[:])
                tps = wp.tile([HID, 128], F32, tag="x1tt")
                nc.scalar.copy(tps[:], tp[:])
                nc.sync.dma_start(aginT[0:16, b * 128:(b + 1) * 128], tps[:])
                nc.sync.dma_start(aginT[16:32, b * 128:(b + 1) * 128], zc[:])
                ad_ps = psp.tile([12, 128], F32, tag="psB")
                nc.tensor.matmul(ad_ps[:], wga[:, 108:120], tps[:],
                                 start=True, stop=True)
                ad_sb = wp.tile([12, 128], F32, tag="adsb")
                nc.scalar.copy(ad_sb[:], ad_ps[:])
                tb = psp.tile([128, 12], F32, tag="psC")
                nc.tensor.transpose(tb[:], ad_sb[:], ident[0:12, 0:12])
                nc.scalar.copy(v3(adr, 12)[:, b, :], tb[:])
            barrier()
            adrows3 = v3(adrows, 12)
            adr3 = v3(adr, 12)
            for ri, r in enumerate(range(1, MAXREG + 1)):
                sz = int(reg_size[r])
                if sz == 0:
                    continue
                rb0, kb0, nblk = int(row_base[ri]) // 128, int(rank_base[ri]) // 128, sz // 128
                for j in range(r):
                    nc.vector.tensor_copy(
                        adrows3[:, rb0 + j * nblk:rb0 + (j + 1) * nblk, :],
                        adr3[:, kb0:kb0 + nblk, :])
            barrier()
            collective("AllGather", OP.bypass, aginT, agoutT)
            # build tab_gat per shard in column-chunks of <=1024 nodes
            for s0 in range(NCORES):
                loc = 0
                while loc < S:
                    cw = min(1024, S - loc)
                    gtiles = cw // 128
                    strip0 = wp.tile([96, 1024], BF16, tag="strip0")
                    strip1 = wp.tile([96, 1024], BF16, tag="strip1")
                    asadT = wp.tile([24, 1024], F32, tag="asadT")
                    nsub = -(-cw // 512)
                    for ci in range(nsub):
                        sc0 = ci * 512
                        scw = min(512, cw - sc0)
                        xr = wp.tile([16, 512], F32, tag="xr")
                        nc.sync.dma_start(
                            xr[:, 0:scw],
                            agoutT[32 * s0:32 * s0 + 16, loc + sc0:loc + sc0 + scw])
                        pA = psp.tile([120, 512], F32, tag="psA")
                        nc.tensor.matmul(pA[:, 0:scw], wga[:], xr[:, 0:scw],
                                         start=True, stop=True)
                        nc.scalar.copy(strip0[:, sc0:sc0 + scw], pA[0:96, 0:scw])
                        nc.scalar.copy(asadT[:, sc0:sc0 + scw], pA[96:120, 0:scw])
                        pB = psp.tile([96, 512], F32, tag="psB")
                        nc.tensor.matmul(pB[:, 0:scw], wgb[:], xr[:, 0:scw],
                                         start=True, stop=True)
                        nc.scalar.copy(strip1[:, sc0:sc0 + scw], pB[:, 0:scw])
                    rowblk = wp.tile([128, 8 * AELEM], F32, tag="rowblk")
                    rb3 = rowblk[:].rearrange("p (g e) -> p g e", e=AELEM)
                    nc.vector.memset(rowblk[:], 0.0)
                    hh0 = rb3[:, 0:gtiles, 16:64].bitcast(BF16)
                    nc.sync.dma_start(hh0, strip0[:, 0:gtiles * 128], transpose=True)
                    hh1 = rb3[:, 0:gtiles, 64:112].bitcast(BF16)
                    nc.sync.dma_start(hh1, strip1[:, 0:gtiles * 128], transpose=True)
                    for g in range(gtiles):
                        pt = psp.tile([128, 24], F32, tag="psC")
                        nc.tensor.transpose(
                            pt[:], asadT[:, g * 128:(g + 1) * 128],
                            ident[0:24, 0:24])
                        nc.scalar.copy(rb3[:, g, 0:12], pt[:, 0:12])
                    c0 = s0 * S + loc
                    nc.sync.dma_start(
                        tab_gat[c0:c0 + cw, :].rearrange("(g p) e -> p g e", p=128),
                        rb3[:, 0:gtiles, :])
                    loc += cw
            barrier()

            # ================= GAT gather =================
            grv = gatrows[:].rearrange("p (b c) -> p b c", c=192)
            drv = v3(denrows, 12)
            for ch in range(NCH):
                gs = [gp.tile([128, CRB * AELEM], F32, tag=f"agbuf{k}",
                              name=f"agbuf{k}_{ch}") for k in range(K)]
                g4s = [t[:].rearrange("p (b e) -> p b e", e=AELEM) for t in gs]
                for k in range(K):
                    base = k * RPAD + ch * CHUNK_ROWS
                    nc.gpsimd.dma_gather(
                        g4s[k], tab_gat[:],
                        idxs[:, base // 16:(base + CHUNK_ROWS) // 16],
                        num_idxs=CHUNK_ROWS, num_idxs_reg=CHUNK_ROWS,
                        elem_size=AELEM, queue_num=k % 4)
                ex = gp.tile([128, K * CRB * 12], F32, tag="ex")
                exv = ex[:].rearrange("p (k b h) -> p k b h", k=K, h=12)
                for k in range(K):
                    nc.vector.tensor_tensor(
                        exv[:, k], g4s[k][:, :, 0:12],
                        adrows3[:, ch * CRB:(ch + 1) * CRB, :], op=OP.add)
                lk = gp.tile([128, K * CRB * 12], F32, tag="lk")
                lkv = lk[:].rearrange("p (k b h) -> p k b h", k=K, h=12)
                nc.scalar.mul(lkv, exv, 0.2)
                nc.vector.tensor_tensor(exv, exv, lkv, op=OP.max)
                nc.scalar.activation(exv, exv, AF.Exp)
                nc.vector.tensor_tensor(lkv[:, 0:2], exv[:, 0:2], exv[:, 2:4], op=OP.add)
                nc.vector.tensor_tensor(
                    drv[:, ch * CRB:(ch + 1) * CRB, :].unsqueeze(1),
                    lkv[:, 0:1], lkv[:, 1:2], op=OP.add)
                exb = gp.tile([128, K * CRB * 12], BF16, tag="exb")
                exbv = exb[:].rearrange("p (k b h) -> p k b h", k=K, h=12)
                nc.vector.tensor_copy(exbv, exv)
                hhs = [g4s[k][:, :, 16:112].bitcast(BF16).rearrange(
                    "p b (h c) -> p b h c", c=HID) for k in range(K)]
                for k in range(K):
                    nc.vector.tensor_tensor(
                        hhs[k], hhs[k],
                        exbv[:, k].unsqueeze(3).broadcast_to([128, CRB, 12, HID]),
                        op=OP.mult)
                hhf = [h.rearrange("p b h c -> p b (h c)") for h in hhs]
                for k in range(2):
                    nc.vector.tensor_tensor(hhf[k], hhf[k], hhf[k + 2], op=OP.add)
                nc.vector.tensor_tensor(
                    grv[:, ch * CRB:(ch + 1) * CRB, :], hhf[0], hhf[1], op=OP.add)
            barrier()

            # ================= GAT epilogue =================
            num = wp.tile([128, SB * 192], F32, tag="num")
            num3 = v3(num, 192)
            combine_rows(grv, num3)
            den = wp.tile([128, SB * 12], F32, tag="den")
            den3 = v3(den, 12)
            combine_rows(drv, den3)
            nc.vector.reciprocal(den3, den3)
            num4 = num3.rearrange("p b (h c) -> p b h c", c=HID)
            nc.vector.tensor_tensor(
                num4, num4,
                den3.unsqueeze(3).broadcast_to([128, SB, 12, HID]), op=OP.mult)
            nc.vector.tensor_tensor(
                num3, num3, bgt[:].unsqueeze(1).broadcast_to([128, SB, 192]),
                op=OP.add)
            el1 = wp.tile([128, SB * 192], F32, tag="el1")
            el13 = v3(el1, 192)
            nc.vector.tensor_scalar_min(el13, num3, 0.0)
            nc.scalar.activation(el13, el13, AF.Exp)
            nc.scalar.activation(num3, num3, AF.Relu)
            nc.vector.tensor_tensor(num3, num3, el13, op=OP.add)
            nc.vector.tensor_scalar_add(num3, num3, -1.0)
            nc.vector.tensor_tensor(
                num3, num3, dinv[:].unsqueeze(2).broadcast_to([128, SB, 192]),
                op=OP.mult)
            barrier()

            # ================= GCN2 =================
            h2n = wp.tile([128, SB * HID], F32, tag="hn")
            project(v3(h2n, HID), num3,
                    [(w2a[:], 0, 96), (w2b[:], 96, 96)], HID)
            barrier()
            ag_and_table(v3(h2n, HID))
            gcn_gather_pass()
            gcn_epilogue(v3(x3n, HID), 16)

            # ================= GCN3 =================
            x3sc = wp.tile([128, SB * HID], F32, tag="x3sc")
            nc.vector.tensor_tensor(
                v3(x3sc, HID), v3(x3n, HID),
                dinv[:].unsqueeze(2).broadcast_to([128, SB, HID]), op=OP.mult)
            h3n = wp.tile([128, SB * HID], F32, tag="hn")
            project(v3(h3n, HID), v3(x3sc, HID), [(w3[:], 0, HID)], HID)
            barrier()
            ag_and_table(v3(h3n, HID))
            gcn_gather_pass()
            gcn_epilogue(v3(x4n, HID), 32, residual=v3(x3n, HID))

            # ================= mean pool + linear =================
            x4m = wp.tile([128, SB * HID], F32, tag="x4m")
            nc.vector.tensor_tensor(
                v3(x4m, HID), v3(x4n, HID),
                mask[:].unsqueeze(2).broadcast_to([128, SB, HID]), op=OP.mult)
            ones = pp.tile([128, 1], F32)
            nc.vector.memset(ones[:], 1.0)
            pool_ps = psp.tile([HID, 1], F32, tag="psB")
            for b in range(SB):
                nc.tensor.matmul(
                    pool_ps[:], v3(x4m, HID)[:, b, :], ones[:],
                    start=(b == 0), stop=(b == SB - 1))
            pool = wp.tile([HID, 1], F32, tag="pool_sb")
            nc.scalar.copy(pool[:], pool_ps[:])
            barrier()
            with tc.tile_critical():
                nc.gpsimd.dma_start(arin[:], pool[:]).then_inc(io_sem, 16)
                st["io"] += 16
                nc.gpsimd.wait_ge(io_sem, st["io"])
                nc.gpsimd.collective_compute(
                    "AllReduce", OP.add, replica_groups=[list(range(NCORES))],
                    ins=[arin[:]], outs=[arout[:]],
                ).then_inc(cc_sem)
                st["cc"] += 1
                nc.gpsimd.wait_ge(cc_sem, st["cc"])
            barrier()
            poolg = wp.tile([HID, 1], F32, tag="poolg")
            nc.sync.dma_start(poolg[:], arout[:])
            nc.scalar.mul(poolg[:], poolg[:], 1.0 / N)
            out_ps = psp.tile([1, OUT], F32, tag="psB")
            nc.tensor.matmul(out_ps[:], poolg[:], wlin[:], start=True, stop=True)
            outt = wp.tile([1, OUT], F32, tag="outt")
            nc.scalar.copy(outt[:], out_ps[:])
            nc.vector.tensor_tensor(outt[:], outt[:], biasb[0:1, 48:48 + OUT],
                                    op=OP.add)
            nc.sync.dma_start(out_ext[:], outt[:])

    nc.compile()
    return nc


# ---------------------------------------------------------------- entry point
def _make_in_maps(inputs, cores, meta):
    x = np.asarray(inputs["x"], np.float32)
    S = meta["SHARDR"]
    W1 = np.asarray(inputs["W1"], np.float32)
    Wg = np.asarray(inputs["Wg"], np.float32)
    att_src = np.asarray(inputs["att_src"], np.float32)
    att_dst = np.asarray(inputs["att_dst"], np.float32)
    W2 = np.asarray(inputs["W2"], np.float32)
    W3 = np.asarray(inputs["W3"], np.float32)
    Wlin = np.asarray(inputs["Wlin"], np.float32)

    Wg3 = Wg.reshape(16, 12, 16)
    Was = np.einsum("khc,hc->kh", Wg3, att_src).astype(np.float32)
    Wad = np.einsum("khc,hc->kh", Wg3, att_dst).astype(np.float32)
    wga = np.concatenate([Wg[:, 0:96], Was, Wad], axis=1)  # [16, 120]
    wgb = np.ascontiguousarray(Wg[:, 96:192])

    biases = np.zeros((1, 256), np.float32)
    biases[0, 0:16] = np.asarray(inputs["b1"], np.float32)
    biases[0, 16:32] = np.asarray(inputs["b2"], np.float32)
    biases[0, 32:48] = np.asarray(inputs["b3"], np.float32)
    biases[0, 48:80] = np.asarray(inputs["blin"], np.float32)
    bg_row = np.asarray(inputs["bg"], np.float32)[None, :]
    padrow = np.zeros((1, AELEM), np.float32)
    padrow[0, 0:12] = -200.0
    ident = np.eye(128, dtype=np.float32)

    xsq = x.reshape(N, F_IN)
    dinv = meta["dinv"]
    in_maps = []
    for c in range(NCORES):
        nl = cores[c]["nodes_local"]
        real = nl >= 0
        xs = np.zeros((S, F_IN), np.float32)
        xs[real] = xsq[nl[real]]
        dv = np.zeros(S, np.float32)
        dv[real] = dinv[nl[real]]
        in_maps.append({
            "xs": xs,
            "dinv": np.ascontiguousarray(_to_pb(dv[:, None], S), dtype=np.float32),
            "mask": np.ascontiguousarray(_to_pb(cores[c]["mask"][:, None], S), dtype=np.float32),
            "idxs": _wrap_idxs(cores[c]["slot_idx"]),
            "w1": W1, "wga": wga, "wgb": wgb,
            "w2a": np.ascontiguousarray(W2[0:96]),
            "w2b": np.ascontiguousarray(W2[96:192]),
            "w3": W3, "wlin": Wlin,
            "biases": biases, "bg": bg_row, "padrow": padrow, "ident": ident,
        })
    return in_maps


def run(inputs, trace=False):
    edge_index = np.asarray(inputs["edge_index"])
    cores, meta = _preprocess(edge_index)
    in_maps = _make_in_maps(inputs, cores, meta)
    nc = _build(meta)
    res = run_bass_kernel_spmd(nc, in_maps, list(range(NCORES)), trace=trace)
    return res


def kernel(**inputs):
    return run(inputs).results[0]["out"]


if __name__ == "__main__":
    import reference
    inputs = reference.setup_inputs()
    inputs = {k: np.asarray(v) for k, v in inputs.items()}
    got = kernel(**inputs)
    exp = np.asarray(reference.reference(**inputs))
    rel = np.abs(got - exp).max() / np.abs(exp).max()
    print("rel err:", rel)


# revision 20
# speedup vs baseline: 1.1516x; 1.0149x over previous
"""Trainium2 Bass kernel for nn_EnhancedGCN (GCN -> GAT -> GCN -> GCN -> mean -> linear).

Strategy (8 NeuronCores, dst-sharded message passing):
- Host: add self loops, in-degree -> dinv (compile-time), relabel nodes
  (core = n % 8, rank grouped by region r = ceil(indeg/K), K=8 slots/row,
  region sizes padded uniform across cores). Slot i = k*RPAD + row holds the
  table index of the k-th in-edge source of `row`; gathered slot i lands at
  SBUF [i%128, i//128], i.e. rows are partition-wrapped and k is a free-dim
  plane -> K-reduce is log-step DVE adds, region combine is block-offset adds.
- Aggregation passes gather from per-core HBM tables via SWDGE dma_gather
  (chunks of 1024 idxs, 4 queues). GCN tables: [TABN+8, 64] f32 rows (16
  used). GAT table: [TABN+8, 128] f32-word rows = [a_src 12 f32 | pad |
  hh 192 bf16 | pad]; the pad row has a_src=-200 so exp() ~ 0.
- a_src/a_dst attention vectors are folded into Wg on the host (Was/Wad);
  softmax max-subtraction is skipped (mathematically identity).
- Between layers: AllGather of [SHARDR, 16] f32 shards; final mean-pool via
  ones-matmul + AllReduce.
"""
import sys

for _p in ("/opt/trn_rl_repo", "/root/.axon_site/_ro/trn_rl_repo"):
    if _p not in sys.path:
        sys.path.append(_p)

import numpy as np

import concourse.bacc as bacc
import concourse.tile as tile
import concourse.mybir as mybir
from concourse.bass_utils import run_bass_kernel_spmd

F32 = mybir.dt.float32
BF16 = mybir.dt.bfloat16
I16 = mybir.dt.int16
AF = mybir.ActivationFunctionType
OP = mybir.AluOpType

NCORES = 8
K = 4            # slots per row
MAXREG = 16
N = 25000
F_IN = 128
HID = 16
OUT = 32
GELEM = 64       # gcn table row, f32 words
AELEM = 128      # gat table row, f32 words
CHUNK_ROWS = 1024


# ---------------------------------------------------------------- host prep
def _preprocess(edge_index):
    src = np.asarray(edge_index[0], np.int64)
    dst = np.asarray(edge_index[1], np.int64)
    loop = np.arange(N, dtype=np.int64)
    src = np.concatenate([src, loop])
    dst = np.concatenate([dst, loop])
    deg = np.bincount(dst, minlength=N)
    dinv = (1.0 / np.sqrt(deg.astype(np.float64))).astype(np.float32)

    core_of = (np.arange(N) % NCORES).astype(np.int64)
    rows_needed = -(-deg // K)
    assert rows_needed.max() <= MAXREG

    reg_nodes = [[None] * (MAXREG + 1) for _ in range(NCORES)]
    for c in range(NCORES):
        mine = np.where(core_of == c)[0]
        for r in range(1, MAXREG + 1):
            sel = mine[rows_needed[mine] == r]
            reg_nodes[c][r] = sel[np.argsort(-deg[sel], kind="stable")]

    reg_size = np.zeros(MAXREG + 1, np.int64)
    for r in range(1, MAXREG + 1):
        m = max(len(reg_nodes[c][r]) for c in range(NCORES))
        reg_size[r] = -(-m // 128) * 128 if m > 0 else 0
    SHARDR = int(reg_size[1:].sum())
    RTOT = int(sum(r * reg_size[r] for r in range(1, MAXREG + 1)))
    RPAD = -(-RTOT // CHUNK_ROWS) * CHUNK_ROWS
    NSLOT = K * RPAD
    TABN = NCORES * SHARDR
    PADROW = TABN
    assert PADROW < 32768, PADROW

    rank_base = np.cumsum([0] + [int(reg_size[r]) for r in range(1, MAXREG + 1)])
    row_base = np.cumsum([0] + [int(r * reg_size[r]) for r in range(1, MAXREG + 1)])

    rank_of = np.full(N, -1, np.int64)
    for c in range(NCORES):
        for ri, r in enumerate(range(1, MAXREG + 1)):
            nodes = reg_nodes[c][r]
            rank_of[nodes] = rank_base[ri] + np.arange(len(nodes))
    tabpos = core_of * SHARDR + rank_of

    cores = []
    for c in range(NCORES):
        slot_idx = np.full(NSLOT, PADROW, np.int64)
        mask = np.zeros(SHARDR, np.float32)
        nodes_local = np.full(SHARDR, -1, np.int64)
        for ri, r in enumerate(range(1, MAXREG + 1)):
            nodes = reg_nodes[c][r]
            mask[rank_base[ri]:rank_base[ri] + len(nodes)] = 1.0
            nodes_local[rank_base[ri]:rank_base[ri] + len(nodes)] = nodes

        emask = core_of[dst] == c
        es, ed = src[emask], dst[emask]
        order = np.argsort(ed, kind="stable")
        es, ed = es[order], ed[order]
        uniq, start_idx = np.unique(ed, return_index=True)
        pos = np.arange(len(ed)) - start_idx[np.searchsorted(uniq, ed)]
        r_of = rows_needed[ed]
        ri_of = r_of - 1
        j = pos // K
        k = pos % K
        rank_in_reg = rank_of[ed] - rank_base[ri_of]
        row = row_base[ri_of] + j * reg_size[r_of] + rank_in_reg
        slot_idx[k * RPAD + row] = tabpos[es]
        cores.append(dict(slot_idx=slot_idx, mask=mask, nodes_local=nodes_local))

    meta = dict(SHARDR=SHARDR, RPAD=RPAD, NSLOT=NSLOT, TABN=TABN, PADROW=PADROW,
                reg_size=reg_size, rank_base=rank_base, row_base=row_base,
                dinv=dinv, tabpos=tabpos)
    return cores, meta


def _wrap_idxs(idx):
    n = len(idx)
    w = idx.reshape(n // 16, 16).T.astype(np.int16)
    return np.tile(w, (8, 1))


def _to_pb(a, S):
    C = a.shape[1] if a.ndim > 1 else 1
    return a.reshape(S // 128, 128, C).transpose(1, 0, 2).reshape(128, -1)


# ---------------------------------------------------------------- kernel build
def _build(meta):
    S, RPAD, NSLOT, TABN = meta["SHARDR"], meta["RPAD"], meta["NSLOT"], meta["TABN"]
    SB = S // 128
    RB = RPAD // 128
    CRB = CHUNK_ROWS // 128
    NCH = RPAD // CHUNK_ROWS
    reg_size, rank_base, row_base = meta["reg_size"], meta["rank_base"], meta["row_base"]

    nc = bacc.Bacc("TRN2", target_bir_lowering=False, debug=False, num_swdge_queues=4)

    xs_in = nc.declare_dram_parameter("xs", [S, F_IN], F32, isOutput=False)
    dinv_in = nc.declare_dram_parameter("dinv", [128, SB], F32, isOutput=False)
    mask_in = nc.declare_dram_parameter("mask", [128, SB], F32, isOutput=False)
    idxs_in = nc.declare_dram_parameter("idxs", [128, NSLOT // 16], I16, isOutput=False)
    w1_in = nc.declare_dram_parameter("w1", [F_IN, HID], F32, isOutput=False)
    wga_in = nc.declare_dram_parameter("wga", [HID, 120], F32, isOutput=False)
    wgb_in = nc.declare_dram_parameter("wgb", [HID, 96], F32, isOutput=False)
    w2a_in = nc.declare_dram_parameter("w2a", [96, HID], F32, isOutput=False)
    w2b_in = nc.declare_dram_parameter("w2b", [96, HID], F32, isOutput=False)
    w3_in = nc.declare_dram_parameter("w3", [HID, HID], F32, isOutput=False)
    wlin_in = nc.declare_dram_parameter("wlin", [HID, OUT], F32, isOutput=False)
    bias_in = nc.declare_dram_parameter("biases", [1, 256], F32, isOutput=False)
    bg_in = nc.declare_dram_parameter("bg", [1, 192], F32, isOutput=False)
    padrow_in = nc.declare_dram_parameter("padrow", [1, AELEM], F32, isOutput=False)
    ident_in = nc.declare_dram_parameter("ident", [128, 128], F32, isOutput=False)
    out_ext = nc.declare_dram_parameter("out", [1, OUT], F32, isOutput=True)

    tab_gcn = nc.dram_tensor("tab_gcn", [TABN + 8, GELEM], F32)
    tab_gat = nc.dram_tensor("tab_gat", [TABN + 8, AELEM], F32)
    agin = nc.dram_tensor("agin", [S, HID], F32)
    agout = nc.dram_tensor("agout", [TABN, HID], F32, addr_space="Shared")
    aginT = nc.dram_tensor("aginT", [32, S], F32)
    agoutT = nc.dram_tensor("agoutT", [NCORES * 32, S], F32, addr_space="Shared")
    arin = nc.dram_tensor("arin", [HID, 1], F32)
    arout = nc.dram_tensor("arout", [HID, 1], F32, addr_space="Shared")

    cc_sem = nc.alloc_semaphore("ccs")
    io_sem = nc.alloc_semaphore("ios")
    st = {"cc": 0, "io": 0}

    with tile.TileContext(nc) as tc:
        with (
            tc.tile_pool(name="persist", bufs=1) as pp,
            tc.tile_pool(name="work", bufs=1) as wp,
            tc.tile_pool(name="gb", bufs=2) as gp,
            tc.tile_pool(name="ps", bufs=2, space="PSUM") as psp,
        ):
            idxs = pp.tile([128, NSLOT // 16], I16)
            nc.sync.dma_start(idxs[:], idxs_in[:])
            dinv = pp.tile([128, SB], F32)
            nc.sync.dma_start(dinv[:], dinv_in[:])
            mask = pp.tile([128, SB], F32)
            nc.sync.dma_start(mask[:], mask_in[:])
            w1 = pp.tile([F_IN, HID], F32)
            nc.sync.dma_start(w1[:], w1_in[:])
            wga = pp.tile([HID, 120], F32)
            nc.sync.dma_start(wga[:], wga_in[:])
            wgb = pp.tile([HID, 96], F32)
            nc.sync.dma_start(wgb[:], wgb_in[:])
            w2a = pp.tile([96, HID], F32)
            nc.sync.dma_start(w2a[:], w2a_in[:])
            w2b = pp.tile([96, HID], F32)
            nc.sync.dma_start(w2b[:], w2b_in[:])
            w3 = pp.tile([HID, HID], F32)
            nc.sync.dma_start(w3[:], w3_in[:])
            wlin = pp.tile([HID, OUT], F32)
            nc.sync.dma_start(wlin[:], wlin_in[:])
            ident = pp.tile([128, 128], F32)
            nc.sync.dma_start(ident[:], ident_in[:])
            bgt = pp.tile([128, 192], F32)
            nc.sync.dma_start(bgt[:], bg_in[:].partition_broadcast(128).squeeze(1))
            biasb = pp.tile([128, 256], F32)
            nc.sync.dma_start(biasb[:], bias_in[:].partition_broadcast(128).squeeze(1))

            nc.sync.dma_start(tab_gat[TABN:TABN + 1, :], padrow_in[:])
            zrow = pp.tile([1, GELEM], F32)
            nc.vector.memset(zrow[:], 0.0)
            nc.sync.dma_start(tab_gcn[TABN:TABN + 1, :], zrow[:])

            x1n = pp.tile([128, SB * HID], F32)
            x3n = pp.tile([128, SB * HID], F32)
            x4n = pp.tile([128, SB * HID], F32)
            gcnrows = pp.tile([128, RB * HID], F32)
            gatrows = pp.tile([128, RB * 192], BF16)
            denrows = pp.tile([128, RB * 12], F32)
            adrows = pp.tile([128, RB * 12], F32)

            def barrier():
                tc.strict_bb_all_engine_barrier()

            def collective(kind, op, src_dram, dst_dram):
                with tc.tile_critical():
                    nc.gpsimd.collective_compute(
                        kind, op, replica_groups=[list(range(NCORES))],
                        ins=[src_dram[:]], outs=[dst_dram[:]],
                    ).then_inc(cc_sem)
                    st["cc"] += 1
                    nc.gpsimd.wait_ge(cc_sem, st["cc"])
                barrier()

            def v3(t, c):
                return t[:].rearrange("p (b c) -> p b c", c=c)

            # per-tile node-major projection: dst[:, b, :] = src[:, b, :] @ Ws
            def project(dst3d, src3d, wlist, cdim):
                """wlist = [(W_ap, src_off, fdim), ...] summed over chunks."""
                for b in range(SB):
                    mm = psp.tile([cdim, 128], F32, tag="psB")
                    for i, (W, off, fd) in enumerate(wlist):
                        tp = psp.tile([fd, 128], F32, tag="psA")
                        nc.tensor.transpose(
                            tp[:], src3d[:, b, off:off + fd], ident[:])
                        tps = wp.tile([128, 128], F32, tag="ptps")
                        nc.scalar.copy(tps[0:fd, :], tp[:])
                        nc.tensor.matmul(mm[:], W, tps[0:fd, :],
                                         start=(i == 0), stop=(i == len(wlist) - 1))
                    mms = wp.tile([cdim, 128], F32, tag="pmms")
                    nc.scalar.copy(mms[:], mm[:])
                    tb = psp.tile([128, cdim], F32, tag="psC")
                    nc.tensor.transpose(tb[:], mms[:], ident[0:cdim, 0:cdim])
                    nc.scalar.copy(dst3d[:, b, :], tb[:])

            def ag_and_table(hn3d):
                nc.sync.dma_start(
                    agin[:].rearrange("(b p) f -> p b f", p=128), hn3d)
                barrier()
                collective("AllGather", OP.bypass, agin, agout)
                nc.sync.dma_start(tab_gcn[0:TABN, 0:HID], agout[:])
                barrier()

            def gcn_gather_pass():
                rows16 = v3(gcnrows, HID)
                for ch in range(NCH):
                    gs = [gp.tile([128, CRB * GELEM], F32, tag=f"gbuf{k}",
                                  name=f"gbuf{k}_{ch}") for k in range(K)]
                    g4s = [t[:].rearrange("p (b e) -> p b e", e=GELEM) for t in gs]
                    for k in range(K):
                        base = k * RPAD + ch * CHUNK_ROWS
                        nc.gpsimd.dma_gather(
                            g4s[k], tab_gcn[:],
                            idxs[:, base // 16:(base + CHUNK_ROWS) // 16],
                            num_idxs=CHUNK_ROWS, num_idxs_reg=CHUNK_ROWS,
                            elem_size=GELEM, queue_num=k % 4)
                    t2 = gp.tile([128, 2 * CRB * HID], F32, tag="gcn2")
                    t2v = t2[:].rearrange("p (k b c) -> p k b c", k=2, c=HID)
                    for k in range(2):
                        nc.vector.tensor_tensor(
                            t2v[:, k], g4s[k][:, :, 0:HID], g4s[k + 2][:, :, 0:HID],
                            op=OP.add)
                    nc.vector.tensor_tensor(
                        rows16[:, ch * CRB:(ch + 1) * CRB, :].unsqueeze(1),
                        t2v[:, 0:1], t2v[:, 1:2], op=OP.add)
                barrier()

            def combine_rows(rows3d, out3d):
                for ri, r in enumerate(range(1, MAXREG + 1)):
                    sz = int(reg_size[r])
                    if sz == 0:
                        continue
                    rb0 = int(row_base[ri]) // 128
                    kb0 = int(rank_base[ri]) // 128
                    nblk = sz // 128
                    dst = out3d[:, kb0:kb0 + nblk, :]
                    if r == 1:
                        nc.vector.tensor_copy(dst, rows3d[:, rb0:rb0 + nblk, :])
                    else:
                        nc.vector.tensor_tensor(
                            dst, rows3d[:, rb0:rb0 + nblk, :],
                            rows3d[:, rb0 + nblk:rb0 + 2 * nblk, :], op=OP.add)
                        for j in range(2, r):
                            nc.vector.tensor_tensor(
                                dst, dst,
                                rows3d[:, rb0 + j * nblk:rb0 + (j + 1) * nblk, :],
                                op=OP.add)

            def gcn_epilogue(xdst3d, bias_off, residual=None):
                agg = wp.tile([128, SB * HID], F32, tag="agg")
                agg3 = v3(agg, HID)
                combine_rows(v3(gcnrows, HID), agg3)
                nc.vector.tensor_tensor(
                    agg3, agg3, dinv[:].unsqueeze(2).broadcast_to([128, SB, HID]),
                    op=OP.mult)
                nc.vector.tensor_tensor(
                    agg3, agg3,
                    biasb[:, bias_off:bias_off + HID].unsqueeze(1).broadcast_to(
                        [128, SB, HID]), op=OP.add)
                nc.scalar.activation(xdst3d, agg3, AF.Relu)
                if residual is not None:
                    nc.vector.tensor_tensor(xdst3d, xdst3d, residual, op=OP.add)
                barrier()

            # ================= GCN1 =================
            xsc = wp.tile([128, SB * F_IN], F32, tag="num")
            nc.sync.dma_start(
                v3(xsc, F_IN), xs_in[:].rearrange("(b p) f -> p b f", p=128))
            nc.vector.tensor_tensor(
                v3(xsc, F_IN), v3(xsc, F_IN),
                dinv[:].unsqueeze(2).broadcast_to([128, SB, F_IN]), op=OP.mult)
            h1n = wp.tile([128, SB * HID], F32, tag="hn")
            project(v3(h1n, HID), v3(xsc, F_IN), [(w1[:], 0, F_IN)], HID)
            barrier()
            ag_and_table(v3(h1n, HID))
            gcn_gather_pass()
            gcn_epilogue(v3(x1n, HID), 0)

            # ================= GAT prep =================
            # per-tile: x1T tile -> aginT cols; adT tile -> adr
            adr = wp.tile([128, SB * 12], F32, tag="adr")
            zc = wp.tile([16, 128], F32, tag="zc")
            nc.vector.memset(zc[:], 0.0)
            for b in range(SB):
                tp = psp.tile([HID, 128], F32, tag="psA")
                nc.tensor.transpose(tp[:], v3(x1n, HID)[:, b, :], ident[:])
                tps = wp.tile([HID, 128], F32, tag="x1tt")
                nc.scalar.copy(tps[:], tp[:])
                nc.sync.dma_start(aginT[0:16, b * 128:(b + 1) * 128], tps[:])
                nc.sync.dma_start(aginT[16:32, b * 128:(b + 1) * 128], zc[:])
                ad_ps = psp.tile([12, 128], F32, tag="psB")
                nc.tensor.matmul(ad_ps[:], wga[:, 108:120], tps[:],
                                 start=True, stop=True)
                ad_sb = wp.tile([12, 128], F32, tag="adsb")
                nc.scalar.copy(ad_sb[:], ad_ps[:])
                tb = psp.tile([128, 12], F32, tag="psC")
                nc.tensor.transpose(tb[:], ad_sb[:], ident[0:12, 0:12])
                nc.scalar.copy(v3(adr, 12)[:, b, :], tb[:])
            barrier()
            adrows3 = v3(adrows, 12)
            adr3 = v3(adr, 12)
            for ri, r in enumerate(range(1, MAXREG + 1)):
                sz = int(reg_size[r])
                if sz == 0:
                    continue
                rb0, kb0, nblk = int(row_base[ri]) // 128, int(rank_base[ri]) // 128, sz // 128
                for j in range(r):
                    nc.vector.tensor_copy(
                        adrows3[:, rb0 + j * nblk:rb0 + (j + 1) * nblk, :],
                        adr3[:, kb0:kb0 + nblk, :])
            barrier()
            collective("AllGather", OP.bypass, aginT, agoutT)
            # build tab_gat per shard in column-chunks of <=1024 nodes
            for s0 in range(NCORES):
                loc = 0
                while loc < S:
                    cw = min(1024, S - loc)
                    gtiles = cw // 128
                    strip0 = wp.tile([96, 1024], BF16, tag="strip0")
                    strip1 = wp.tile([96, 1024], BF16, tag="strip1")
                    asadT = wp.tile([24, 1024], F32, tag="asadT")
                    nsub = -(-cw // 512)
                    for ci in range(nsub):
                        sc0 = ci * 512
                        scw = min(512, cw - sc0)
                        xr = wp.tile([16, 512], F32, tag="xr")
                        nc.sync.dma_start(
                            xr[:, 0:scw],
                            agoutT[32 * s0:32 * s0 + 16, loc + sc0:loc + sc0 + scw])
                        pA = psp.tile([120, 512], F32, tag="psA")
                        nc.tensor.matmul(pA[:, 0:scw], wga[:], xr[:, 0:scw],
                                         start=True, stop=True)
                        nc.scalar.copy(strip0[:, sc0:sc0 + scw], pA[0:96, 0:scw])
                        nc.scalar.copy(asadT[:, sc0:sc0 + scw], pA[96:120, 0:scw])
                        pB = psp.tile([96, 512], F32, tag="psB")
                        nc.tensor.matmul(pB[:, 0:scw], wgb[:], xr[:, 0:scw],
                                         start=True, stop=True)
                        nc.scalar.copy(strip1[:, sc0:sc0 + scw], pB[:, 0:scw])
                    rowblk = wp.tile([128, 8 * AELEM], F32, tag="rowblk")
                    rb3 = rowblk[:].rearrange("p (g e) -> p g e", e=AELEM)
                    nc.vector.memset(rowblk[:], 0.0)
                    hh0 = rb3[:, 0:gtiles, 16:64].bitcast(BF16)
                    nc.sync.dma_start(hh0, strip0[:, 0:gtiles * 128], transpose=True)
                    hh1 = rb3[:, 0:gtiles, 64:112].bitcast(BF16)
                    nc.sync.dma_start(hh1, strip1[:, 0:gtiles * 128], transpose=True)
                    for g in range(gtiles):
                        pt = psp.tile([128, 24], F32, tag="psC")
                        nc.tensor.transpose(
                            pt[:], asadT[:, g * 128:(g + 1) * 128],
                            ident[0:24, 0:24])
                        nc.scalar.copy(rb3[:, g, 0:12], pt[:, 0:12])
                    c0 = s0 * S + loc
                    nc.sync.dma_start(
                        tab_gat[c0:c0 + cw, :].rearrange("(g p) e -> p g e", p=128),
                        rb3[:, 0:gtiles, :])
                    loc += cw
            barrier()

            # ================= GAT gather =================
            grv = gatrows[:].rearrange("p (b c) -> p b c", c=192)
            drv = v3(denrows, 12)
            for ch in range(NCH):
                gs = [gp.tile([128, CRB * AELEM], F32, tag=f"agbuf{k}",
                              name=f"agbuf{k}_{ch}") for k in range(K)]
                g4s = [t[:].rearrange("p (b e) -> p b e", e=AELEM) for t in gs]
                for k in range(K):
                    base = k * RPAD + ch * CHUNK_ROWS
                    nc.gpsimd.dma_gather(
                        g4s[k], tab_gat[:],
                        idxs[:, base // 16:(base + CHUNK_ROWS) // 16],
                        num_idxs=CHUNK_ROWS, num_idxs_reg=CHUNK_ROWS,
                        elem_size=AELEM, queue_num=k % 4)
                ex = gp.tile([128, K * CRB * 12], F32, tag="ex")
                exv = ex[:].rearrange("p (k b h) -> p k b h", k=K, h=12)
                for k in range(K):
                    nc.vector.tensor_tensor(
                        exv[:, k], g4s[k][:, :, 0:12],
                        adrows3[:, ch * CRB:(ch + 1) * CRB, :], op=OP.add)
                lk = gp.tile([128, K * CRB * 12], F32, tag="lk")
                lkv = lk[:].rearrange("p (k b h) -> p k b h", k=K, h=12)
                nc.scalar.mul(lkv, exv, 0.2)
                nc.vector.tensor_tensor(exv, exv, lkv, op=OP.max)
                nc.scalar.activation(exv, exv, AF.Exp)
                nc.vector.tensor_tensor(lkv[:, 0:2], exv[:, 0:2], exv[:, 2:4], op=OP.add)
                nc.vector.tensor_tensor(
                    drv[:, ch * CRB:(ch + 1) * CRB, :].unsqueeze(1),
                    lkv[:, 0:1], lkv[:, 1:2], op=OP.add)
                exb = gp.tile([128, K * CRB * 12], BF16, tag="exb")
                exbv = exb[:].rearrange("p (k b h) -> p k b h", k=K, h=12)
                nc.vector.tensor_copy(exbv, exv)
                hhs = [g4s[k][:, :, 16:112].bitcast(BF16).rearrange(
                    "p b (h c) -> p b h c", c=HID) for k in range(K)]
                for k in range(K):
                    nc.vector.tensor_tensor(
                        hhs[k], hhs[k],
                        exbv[:, k].unsqueeze(3).broadcast_to([128, CRB, 12, HID]),
                        op=OP.mult)
                hhf = [h.rearrange("p b h c -> p b (h c)") for h in hhs]
                for k in range(2):
                    nc.vector.tensor_tensor(hhf[k], hhf[k], hhf[k + 2], op=OP.add)
                nc.vector.tensor_tensor(
                    grv[:, ch * CRB:(ch + 1) * CRB, :], hhf[0], hhf[1], op=OP.add)
            barrier()

            # ================= GAT epilogue =================
            num = wp.tile([128, SB * 192], F32, tag="num")
            num3 = v3(num, 192)
            combine_rows(grv, num3)
            den = wp.tile([128, SB * 12], F32, tag="den")
            den3 = v3(den, 12)
            combine_rows(drv, den3)
            nc.vector.reciprocal(den3, den3)
            num4 = num3.rearrange("p b (h c) -> p b h c", c=HID)
            nc.vector.tensor_tensor(
                num4, num4,
                den3.unsqueeze(3).broadcast_to([128, SB, 12, HID]), op=OP.mult)
            nc.vector.tensor_tensor(
                num3, num3, bgt[:].unsqueeze(1).broadcast_to([128, SB, 192]),
                op=OP.add)
            for b in range(SB):
                el1 = wp.tile([128, 192], F32, tag="el1", name=f"el1_{b}")
                nc.vector.tensor_scalar_min(el1[:], num3[:, b, :], 0.0)
                nc.scalar.activation(el1[:], el1[:], AF.Exp)
                nc.scalar.activation(num3[:, b, :], num3[:, b, :], AF.Relu)
                nc.vector.tensor_tensor(num3[:, b, :], num3[:, b, :], el1[:],
                                        op=OP.add)
                nc.vector.tensor_scalar_add(num3[:, b, :], num3[:, b, :], -1.0)
            nc.vector.tensor_tensor(
                num3, num3, dinv[:].unsqueeze(2).broadcast_to([128, SB, 192]),
                op=OP.mult)
            barrier()

            # ================= GCN2 =================
            h2n = wp.tile([128, SB * HID], F32, tag="hn")
            project(v3(h2n, HID), num3,
                    [(w2a[:], 0, 96), (w2b[:], 96, 96)], HID)
            barrier()
            ag_and_table(v3(h2n, HID))
            gcn_gather_pass()
            gcn_epilogue(v3(x3n, HID), 16)

            # ================= GCN3 =================
            x3sc = wp.tile([128, SB * HID], F32, tag="x3sc")
            nc.vector.tensor_tensor(
                v3(x3sc, HID), v3(x3n, HID),
                dinv[:].unsqueeze(2).broadcast_to([128, SB, HID]), op=OP.mult)
            h3n = wp.tile([128, SB * HID], F32, tag="hn")
            project(v3(h3n, HID), v3(x3sc, HID), [(w3[:], 0, HID)], HID)
            barrier()
            ag_and_table(v3(h3n, HID))
            gcn_gather_pass()
            gcn_epilogue(v3(x4n, HID), 32, residual=v3(x3n, HID))

            # ================= mean pool + linear =================
            x4m = wp.tile([128, SB * HID], F32, tag="x4m")
            nc.vector.tensor_tensor(
                v3(x4m, HID), v3(x4n, HID),
                mask[:].unsqueeze(2).broadcast_to([128, SB, HID]), op=OP.mult)
            ones = pp.tile([128, 1], F32)
            nc.vector.memset(ones[:], 1.0)
            pool_ps = psp.tile([HID, 1], F32, tag="psB")
            for b in range(SB):
                nc.tensor.matmul(
                    pool_ps[:], v3(x4m, HID)[:, b, :], ones[:],
                    start=(b == 0), stop=(b == SB - 1))
            pool = wp.tile([HID, 1], F32, tag="pool_sb")
            nc.scalar.copy(pool[:], pool_ps[:])
            barrier()
            with tc.tile_critical():
                nc.gpsimd.dma_start(arin[:], pool[:]).then_inc(io_sem, 16)
                st["io"] += 16
                nc.gpsimd.wait_ge(io_sem, st["io"])
                nc.gpsimd.collective_compute(
                    "AllReduce", OP.add, replica_groups=[list(range(NCORES))],
                    ins=[arin[:]], outs=[arout[:]],
                ).then_inc(cc_sem)
                st["cc"] += 1
                nc.gpsimd.wait_ge(cc_sem, st["cc"])
            barrier()
            poolg = wp.tile([HID, 1], F32, tag="poolg")
            nc.sync.dma_start(poolg[:], arout[:])
            nc.scalar.mul(poolg[:], poolg[:], 1.0 / N)
            out_ps = psp.tile([1, OUT], F32, tag="psB")
            nc.tensor.matmul(out_ps[:], poolg[:], wlin[:], start=True, stop=True)
            outt = wp.tile([1, OUT], F32, tag="outt")
            nc.scalar.copy(outt[:], out_ps[:])
            nc.vector.tensor_tensor(outt[:], outt[:], biasb[0:1, 48:48 + OUT],
                                    op=OP.add)
            nc.sync.dma_start(out_ext[:], outt[:])

    nc.compile()
    return nc


# ---------------------------------------------------------------- entry point
def _make_in_maps(inputs, cores, meta):
    x = np.asarray(inputs["x"], np.float32)
    S = meta["SHARDR"]
    W1 = np.asarray(inputs["W1"], np.float32)
    Wg = np.asarray(inputs["Wg"], np.float32)
    att_src = np.asarray(inputs["att_src"], np.float32)
    att_dst = np.asarray(inputs["att_dst"], np.float32)
    W2 = np.asarray(inputs["W2"], np.float32)
    W3 = np.asarray(inputs["W3"], np.float32)
    Wlin = np.asarray(inputs["Wlin"], np.float32)

    Wg3 = Wg.reshape(16, 12, 16)
    Was = np.einsum("khc,hc->kh", Wg3, att_src).astype(np.float32)
    Wad = np.einsum("khc,hc->kh", Wg3, att_dst).astype(np.float32)
    wga = np.concatenate([Wg[:, 0:96], Was, Wad], axis=1)  # [16, 120]
    wgb = np.ascontiguousarray(Wg[:, 96:192])

    biases = np.zeros((1, 256), np.float32)
    biases[0, 0:16] = np.asarray(inputs["b1"], np.float32)
    biases[0, 16:32] = np.asarray(inputs["b2"], np.float32)
    biases[0, 32:48] = np.asarray(inputs["b3"], np.float32)
    biases[0, 48:80] = np.asarray(inputs["blin"], np.float32)
    bg_row = np.asarray(inputs["bg"], np.float32)[None, :]
    padrow = np.zeros((1, AELEM), np.float32)
    padrow[0, 0:12] = -200.0
    ident = np.eye(128, dtype=np.float32)

    xsq = x.reshape(N, F_IN)
    dinv = meta["dinv"]
    in_maps = []
    for c in range(NCORES):
        nl = cores[c]["nodes_local"]
        real = nl >= 0
        xs = np.zeros((S, F_IN), np.float32)
        xs[real] = xsq[nl[real]]
        dv = np.zeros(S, np.float32)
        dv[real] = dinv[nl[real]]
        in_maps.append({
            "xs": xs,
            "dinv": np.ascontiguousarray(_to_pb(dv[:, None], S), dtype=np.float32),
            "mask": np.ascontiguousarray(_to_pb(cores[c]["mask"][:, None], S), dtype=np.float32),
            "idxs": _wrap_idxs(cores[c]["slot_idx"]),
            "w1": W1, "wga": wga, "wgb": wgb,
            "w2a": np.ascontiguousarray(W2[0:96]),
            "w2b": np.ascontiguousarray(W2[96:192]),
            "w3": W3, "wlin": Wlin,
            "biases": biases, "bg": bg_row, "padrow": padrow, "ident": ident,
        })
    return in_maps


def run(inputs, trace=False):
    edge_index = np.asarray(inputs["edge_index"])
    cores, meta = _preprocess(edge_index)
    in_maps = _make_in_maps(inputs, cores, meta)
    nc = _build(meta)
    res = run_bass_kernel_spmd(nc, in_maps, list(range(NCORES)), trace=trace)
    return res


def kernel(**inputs):
    return run(inputs).results[0]["out"]


if __name__ == "__main__":
    import reference
    inputs = reference.setup_inputs()
    inputs = {k: np.asarray(v) for k, v in inputs.items()}
    got = kernel(**inputs)
    exp = np.asarray(reference.reference(**inputs))
    rel = np.abs(got - exp).max() / np.abs(exp).max()
    print("rel err:", rel)


# revision 21
# speedup vs baseline: 1.1723x; 1.0179x over previous
"""Trainium2 Bass kernel for nn_EnhancedGCN (GCN -> GAT -> GCN -> GCN -> mean -> linear).

Strategy (8 NeuronCores, dst-sharded message passing):
- Host: add self loops, in-degree -> dinv (compile-time), relabel nodes
  (core = n % 8, rank grouped by region r = ceil(indeg/K), K=8 slots/row,
  region sizes padded uniform across cores). Slot i = k*RPAD + row holds the
  table index of the k-th in-edge source of `row`; gathered slot i lands at
  SBUF [i%128, i//128], i.e. rows are partition-wrapped and k is a free-dim
  plane -> K-reduce is log-step DVE adds, region combine is block-offset adds.
- Aggregation passes gather from per-core HBM tables via SWDGE dma_gather
  (chunks of 1024 idxs, 4 queues). GCN tables: [TABN+8, 64] f32 rows (16
  used). GAT table: [TABN+8, 128] f32-word rows = [a_src 12 f32 | pad |
  hh 192 bf16 | pad]; the pad row has a_src=-200 so exp() ~ 0.
- a_src/a_dst attention vectors are folded into Wg on the host (Was/Wad);
  softmax max-subtraction is skipped (mathematically identity).
- Between layers: AllGather of [SHARDR, 16] f32 shards; final mean-pool via
  ones-matmul + AllReduce.
"""
import sys

for _p in ("/opt/trn_rl_repo", "/root/.axon_site/_ro/trn_rl_repo"):
    if _p not in sys.path:
        sys.path.append(_p)

import numpy as np

import concourse.bacc as bacc
import concourse.tile as tile
import concourse.mybir as mybir
from concourse.bass_utils import run_bass_kernel_spmd

F32 = mybir.dt.float32
BF16 = mybir.dt.bfloat16
I16 = mybir.dt.int16
AF = mybir.ActivationFunctionType
OP = mybir.AluOpType

NCORES = 8
K = 4            # slots per row
MAXREG = 16
N = 25000
F_IN = 128
HID = 16
OUT = 32
GELEM = 64       # gcn table row, f32 words
AELEM = 128      # gat table row, f32 words
CHUNK_ROWS = 1024


# ---------------------------------------------------------------- host prep
def _preprocess(edge_index):
    src = np.asarray(edge_index[0], np.int64)
    dst = np.asarray(edge_index[1], np.int64)
    loop = np.arange(N, dtype=np.int64)
    src = np.concatenate([src, loop])
    dst = np.concatenate([dst, loop])
    deg = np.bincount(dst, minlength=N)
    dinv = (1.0 / np.sqrt(deg.astype(np.float64))).astype(np.float32)

    core_of = (np.arange(N) % NCORES).astype(np.int64)
    rows_needed = -(-deg // K)
    assert rows_needed.max() <= MAXREG

    reg_nodes = [[None] * (MAXREG + 1) for _ in range(NCORES)]
    for c in range(NCORES):
        mine = np.where(core_of == c)[0]
        for r in range(1, MAXREG + 1):
            sel = mine[rows_needed[mine] == r]
            reg_nodes[c][r] = sel[np.argsort(-deg[sel], kind="stable")]

    reg_size = np.zeros(MAXREG + 1, np.int64)
    for r in range(1, MAXREG + 1):
        m = max(len(reg_nodes[c][r]) for c in range(NCORES))
        reg_size[r] = -(-m // 128) * 128 if m > 0 else 0
    SHARDR = int(reg_size[1:].sum())
    RTOT = int(sum(r * reg_size[r] for r in range(1, MAXREG + 1)))
    RPAD = -(-RTOT // CHUNK_ROWS) * CHUNK_ROWS
    NSLOT = K * RPAD
    TABN = NCORES * SHARDR
    PADROW = TABN
    assert PADROW < 32768, PADROW

    rank_base = np.cumsum([0] + [int(reg_size[r]) for r in range(1, MAXREG + 1)])
    row_base = np.cumsum([0] + [int(r * reg_size[r]) for r in range(1, MAXREG + 1)])

    rank_of = np.full(N, -1, np.int64)
    for c in range(NCORES):
        for ri, r in enumerate(range(1, MAXREG + 1)):
            nodes = reg_nodes[c][r]
            rank_of[nodes] = rank_base[ri] + np.arange(len(nodes))
    tabpos = core_of * SHARDR + rank_of

    cores = []
    for c in range(NCORES):
        slot_idx = np.full(NSLOT, PADROW, np.int64)
        mask = np.zeros(SHARDR, np.float32)
        nodes_local = np.full(SHARDR, -1, np.int64)
        for ri, r in enumerate(range(1, MAXREG + 1)):
            nodes = reg_nodes[c][r]
            mask[rank_base[ri]:rank_base[ri] + len(nodes)] = 1.0
            nodes_local[rank_base[ri]:rank_base[ri] + len(nodes)] = nodes

        emask = core_of[dst] == c
        es, ed = src[emask], dst[emask]
        order = np.argsort(ed, kind="stable")
        es, ed = es[order], ed[order]
        uniq, start_idx = np.unique(ed, return_index=True)
        pos = np.arange(len(ed)) - start_idx[np.searchsorted(uniq, ed)]
        r_of = rows_needed[ed]
        ri_of = r_of - 1
        j = pos // K
        k = pos % K
        rank_in_reg = rank_of[ed] - rank_base[ri_of]
        row = row_base[ri_of] + j * reg_size[r_of] + rank_in_reg
        slot_idx[k * RPAD + row] = tabpos[es]
        cores.append(dict(slot_idx=slot_idx, mask=mask, nodes_local=nodes_local))

    meta = dict(SHARDR=SHARDR, RPAD=RPAD, NSLOT=NSLOT, TABN=TABN, PADROW=PADROW,
                reg_size=reg_size, rank_base=rank_base, row_base=row_base,
                dinv=dinv, tabpos=tabpos)
    return cores, meta


def _wrap_idxs(idx):
    n = len(idx)
    w = idx.reshape(n // 16, 16).T.astype(np.int16)
    return np.tile(w, (8, 1))


def _to_pb(a, S):
    C = a.shape[1] if a.ndim > 1 else 1
    return a.reshape(S // 128, 128, C).transpose(1, 0, 2).reshape(128, -1)


# ---------------------------------------------------------------- kernel build
def _build(meta):
    S, RPAD, NSLOT, TABN = meta["SHARDR"], meta["RPAD"], meta["NSLOT"], meta["TABN"]
    SB = S // 128
    RB = RPAD // 128
    CRB = CHUNK_ROWS // 128
    NCH = RPAD // CHUNK_ROWS
    reg_size, rank_base, row_base = meta["reg_size"], meta["rank_base"], meta["row_base"]

    nc = bacc.Bacc("TRN2", target_bir_lowering=False, debug=False, num_swdge_queues=4)

    xs_in = nc.declare_dram_parameter("xs", [S, F_IN], F32, isOutput=False)
    dinv_in = nc.declare_dram_parameter("dinv", [128, SB], F32, isOutput=False)
    mask_in = nc.declare_dram_parameter("mask", [128, SB], F32, isOutput=False)
    idxs_in = nc.declare_dram_parameter("idxs", [128, NSLOT // 16], I16, isOutput=False)
    w1_in = nc.declare_dram_parameter("w1", [F_IN, HID], F32, isOutput=False)
    wga_in = nc.declare_dram_parameter("wga", [HID, 120], F32, isOutput=False)
    wgb_in = nc.declare_dram_parameter("wgb", [HID, 96], F32, isOutput=False)
    w2a_in = nc.declare_dram_parameter("w2a", [96, HID], F32, isOutput=False)
    w2b_in = nc.declare_dram_parameter("w2b", [96, HID], F32, isOutput=False)
    w3_in = nc.declare_dram_parameter("w3", [HID, HID], F32, isOutput=False)
    wlin_in = nc.declare_dram_parameter("wlin", [HID, OUT], F32, isOutput=False)
    bias_in = nc.declare_dram_parameter("biases", [1, 256], F32, isOutput=False)
    bg_in = nc.declare_dram_parameter("bg", [1, 192], F32, isOutput=False)
    padrow_in = nc.declare_dram_parameter("padrow", [1, AELEM], F32, isOutput=False)
    ident_in = nc.declare_dram_parameter("ident", [128, 128], F32, isOutput=False)
    out_ext = nc.declare_dram_parameter("out", [1, OUT], F32, isOutput=True)

    tab_gcn = nc.dram_tensor("tab_gcn", [TABN + 8, GELEM], F32)
    tab_gat = nc.dram_tensor("tab_gat", [TABN + 8, AELEM], F32)
    agin = nc.dram_tensor("agin", [S, HID], F32)
    agout = nc.dram_tensor("agout", [TABN, HID], F32, addr_space="Shared")
    aginT = nc.dram_tensor("aginT", [32, S], F32)
    agoutT = nc.dram_tensor("agoutT", [NCORES * 32, S], F32, addr_space="Shared")
    arin = nc.dram_tensor("arin", [HID, 1], F32)
    arout = nc.dram_tensor("arout", [HID, 1], F32, addr_space="Shared")

    cc_sem = nc.alloc_semaphore("ccs")
    io_sem = nc.alloc_semaphore("ios")
    st = {"cc": 0, "io": 0}

    with tile.TileContext(nc) as tc:
        with (
            tc.tile_pool(name="persist", bufs=1) as pp,
            tc.tile_pool(name="work", bufs=1) as wp,
            tc.tile_pool(name="gb", bufs=2) as gp,
            tc.tile_pool(name="ps", bufs=2, space="PSUM") as psp,
        ):
            idxs = pp.tile([128, NSLOT // 16], I16)
            nc.sync.dma_start(idxs[:], idxs_in[:])
            dinv = pp.tile([128, SB], F32)
            nc.sync.dma_start(dinv[:], dinv_in[:])
            mask = pp.tile([128, SB], F32)
            nc.sync.dma_start(mask[:], mask_in[:])
            w1 = pp.tile([F_IN, HID], F32)
            nc.sync.dma_start(w1[:], w1_in[:])
            wga = pp.tile([HID, 120], F32)
            nc.sync.dma_start(wga[:], wga_in[:])
            wgb = pp.tile([HID, 96], F32)
            nc.sync.dma_start(wgb[:], wgb_in[:])
            w2a = pp.tile([96, HID], F32)
            nc.sync.dma_start(w2a[:], w2a_in[:])
            w2b = pp.tile([96, HID], F32)
            nc.sync.dma_start(w2b[:], w2b_in[:])
            w3 = pp.tile([HID, HID], F32)
            nc.sync.dma_start(w3[:], w3_in[:])
            wlin = pp.tile([HID, OUT], F32)
            nc.sync.dma_start(wlin[:], wlin_in[:])
            ident = pp.tile([128, 128], F32)
            nc.sync.dma_start(ident[:], ident_in[:])
            bgt = pp.tile([128, 192], F32)
            nc.sync.dma_start(bgt[:], bg_in[:].partition_broadcast(128).squeeze(1))
            biasb = pp.tile([128, 256], F32)
            nc.sync.dma_start(biasb[:], bias_in[:].partition_broadcast(128).squeeze(1))

            nc.sync.dma_start(tab_gat[TABN:TABN + 1, :], padrow_in[:])
            zrow = pp.tile([1, GELEM], F32)
            nc.vector.memset(zrow[:], 0.0)
            nc.sync.dma_start(tab_gcn[TABN:TABN + 1, :], zrow[:])

            x1n = pp.tile([128, SB * HID], F32)
            x3n = pp.tile([128, SB * HID], F32)
            x4n = pp.tile([128, SB * HID], F32)
            gcnrows = pp.tile([128, RB * HID], F32)
            gatrows = pp.tile([128, RB * 192], BF16)
            denrows = pp.tile([128, RB * 12], F32)
            adrows = pp.tile([128, RB * 12], F32)

            def barrier():
                tc.strict_bb_all_engine_barrier()

            def collective(kind, op, src_dram, dst_dram):
                with tc.tile_critical():
                    nc.gpsimd.collective_compute(
                        kind, op, replica_groups=[list(range(NCORES))],
                        ins=[src_dram[:]], outs=[dst_dram[:]],
                    ).then_inc(cc_sem)
                    st["cc"] += 1
                    nc.gpsimd.wait_ge(cc_sem, st["cc"])
                barrier()

            def v3(t, c):
                return t[:].rearrange("p (b c) -> p b c", c=c)

            # per-tile node-major projection: dst[:, b, :] = src[:, b, :] @ Ws
            def project(dst3d, src3d, wlist, cdim):
                """wlist = [(W_ap, src_off, fdim), ...] summed over chunks."""
                for b in range(SB):
                    mm = psp.tile([cdim, 128], F32, tag="psB")
                    for i, (W, off, fd) in enumerate(wlist):
                        tp = psp.tile([fd, 128], F32, tag="psA")
                        nc.tensor.transpose(
                            tp[:], src3d[:, b, off:off + fd], ident[:])
                        tps = wp.tile([128, 128], F32, tag="ptps")
                        nc.scalar.copy(tps[0:fd, :], tp[:])
                        nc.tensor.matmul(mm[:], W, tps[0:fd, :],
                                         start=(i == 0), stop=(i == len(wlist) - 1))
                    mms = wp.tile([cdim, 128], F32, tag="pmms")
                    nc.scalar.copy(mms[:], mm[:])
                    tb = psp.tile([128, cdim], F32, tag="psC")
                    nc.tensor.transpose(tb[:], mms[:], ident[0:cdim, 0:cdim])
                    nc.scalar.copy(dst3d[:, b, :], tb[:])

            def ag_and_table(hn3d):
                nc.sync.dma_start(
                    agin[:].rearrange("(b p) f -> p b f", p=128), hn3d)
                barrier()
                collective("AllGather", OP.bypass, agin, agout)
                nc.sync.dma_start(tab_gcn[0:TABN, 0:HID], agout[:])
                barrier()

            def gcn_gather_pass():
                rows16 = v3(gcnrows, HID)
                for ch in range(NCH):
                    gs = [gp.tile([128, CRB * GELEM], F32, tag=f"gbuf{k}",
                                  name=f"gbuf{k}_{ch}") for k in range(K)]
                    g4s = [t[:].rearrange("p (b e) -> p b e", e=GELEM) for t in gs]
                    for k in range(K):
                        base = k * RPAD + ch * CHUNK_ROWS
                        nc.gpsimd.dma_gather(
                            g4s[k], tab_gcn[:],
                            idxs[:, base // 16:(base + CHUNK_ROWS) // 16],
                            num_idxs=CHUNK_ROWS, num_idxs_reg=CHUNK_ROWS,
                            elem_size=GELEM, queue_num=k % 4)
                    t2 = gp.tile([128, 2 * CRB * HID], F32, tag="gcn2")
                    t2v = t2[:].rearrange("p (k b c) -> p k b c", k=2, c=HID)
                    for k in range(2):
                        nc.vector.tensor_tensor(
                            t2v[:, k], g4s[k][:, :, 0:HID], g4s[k + 2][:, :, 0:HID],
                            op=OP.add)
                    nc.vector.tensor_tensor(
                        rows16[:, ch * CRB:(ch + 1) * CRB, :].unsqueeze(1),
                        t2v[:, 0:1], t2v[:, 1:2], op=OP.add)

            def combine_rows(rows3d, out3d):
                for ri, r in enumerate(range(1, MAXREG + 1)):
                    sz = int(reg_size[r])
                    if sz == 0:
                        continue
                    rb0 = int(row_base[ri]) // 128
                    kb0 = int(rank_base[ri]) // 128
                    nblk = sz // 128
                    dst = out3d[:, kb0:kb0 + nblk, :]
                    if r == 1:
                        nc.vector.tensor_copy(dst, rows3d[:, rb0:rb0 + nblk, :])
                    else:
                        nc.vector.tensor_tensor(
                            dst, rows3d[:, rb0:rb0 + nblk, :],
                            rows3d[:, rb0 + nblk:rb0 + 2 * nblk, :], op=OP.add)
                        for j in range(2, r):
                            nc.vector.tensor_tensor(
                                dst, dst,
                                rows3d[:, rb0 + j * nblk:rb0 + (j + 1) * nblk, :],
                                op=OP.add)

            def gcn_epilogue(xdst3d, bias_off, residual=None):
                agg = wp.tile([128, SB * HID], F32, tag="agg")
                agg3 = v3(agg, HID)
                combine_rows(v3(gcnrows, HID), agg3)
                nc.vector.tensor_tensor(
                    agg3, agg3, dinv[:].unsqueeze(2).broadcast_to([128, SB, HID]),
                    op=OP.mult)
                nc.vector.tensor_tensor(
                    agg3, agg3,
                    biasb[:, bias_off:bias_off + HID].unsqueeze(1).broadcast_to(
                        [128, SB, HID]), op=OP.add)
                nc.scalar.activation(xdst3d, agg3, AF.Relu)
                if residual is not None:
                    nc.vector.tensor_tensor(xdst3d, xdst3d, residual, op=OP.add)

            # ================= GCN1 =================
            xsc = wp.tile([128, SB * F_IN], F32, tag="num")
            nc.sync.dma_start(
                v3(xsc, F_IN), xs_in[:].rearrange("(b p) f -> p b f", p=128))
            nc.vector.tensor_tensor(
                v3(xsc, F_IN), v3(xsc, F_IN),
                dinv[:].unsqueeze(2).broadcast_to([128, SB, F_IN]), op=OP.mult)
            h1n = wp.tile([128, SB * HID], F32, tag="hn")
            project(v3(h1n, HID), v3(xsc, F_IN), [(w1[:], 0, F_IN)], HID)
            ag_and_table(v3(h1n, HID))
            gcn_gather_pass()
            gcn_epilogue(v3(x1n, HID), 0)

            # ================= GAT prep =================
            # per-tile: x1T tile -> aginT cols; adT tile -> adr
            adr = wp.tile([128, SB * 12], F32, tag="adr")
            zc = wp.tile([16, 128], F32, tag="zc")
            nc.vector.memset(zc[:], 0.0)
            for b in range(SB):
                tp = psp.tile([HID, 128], F32, tag="psA")
                nc.tensor.transpose(tp[:], v3(x1n, HID)[:, b, :], ident[:])
                tps = wp.tile([HID, 128], F32, tag="x1tt")
                nc.scalar.copy(tps[:], tp[:])
                nc.sync.dma_start(aginT[0:16, b * 128:(b + 1) * 128], tps[:])
                nc.sync.dma_start(aginT[16:32, b * 128:(b + 1) * 128], zc[:])
                ad_ps = psp.tile([12, 128], F32, tag="psB")
                nc.tensor.matmul(ad_ps[:], wga[:, 108:120], tps[:],
                                 start=True, stop=True)
                ad_sb = wp.tile([12, 128], F32, tag="adsb")
                nc.scalar.copy(ad_sb[:], ad_ps[:])
                tb = psp.tile([128, 12], F32, tag="psC")
                nc.tensor.transpose(tb[:], ad_sb[:], ident[0:12, 0:12])
                nc.scalar.copy(v3(adr, 12)[:, b, :], tb[:])
            barrier()
            adrows3 = v3(adrows, 12)
            adr3 = v3(adr, 12)
            for ri, r in enumerate(range(1, MAXREG + 1)):
                sz = int(reg_size[r])
                if sz == 0:
                    continue
                rb0, kb0, nblk = int(row_base[ri]) // 128, int(rank_base[ri]) // 128, sz // 128
                for j in range(r):
                    nc.vector.tensor_copy(
                        adrows3[:, rb0 + j * nblk:rb0 + (j + 1) * nblk, :],
                        adr3[:, kb0:kb0 + nblk, :])
            barrier()
            collective("AllGather", OP.bypass, aginT, agoutT)
            # build tab_gat per shard in column-chunks of <=1024 nodes
            for s0 in range(NCORES):
                loc = 0
                while loc < S:
                    cw = min(1024, S - loc)
                    gtiles = cw // 128
                    strip0 = wp.tile([96, 1024], BF16, tag="strip0")
                    strip1 = wp.tile([96, 1024], BF16, tag="strip1")
                    asadT = wp.tile([24, 1024], F32, tag="asadT")
                    nsub = -(-cw // 512)
                    for ci in range(nsub):
                        sc0 = ci * 512
                        scw = min(512, cw - sc0)
                        xr = wp.tile([16, 512], F32, tag="xr")
                        nc.sync.dma_start(
                            xr[:, 0:scw],
                            agoutT[32 * s0:32 * s0 + 16, loc + sc0:loc + sc0 + scw])
                        pA = psp.tile([120, 512], F32, tag="psA")
                        nc.tensor.matmul(pA[:, 0:scw], wga[:], xr[:, 0:scw],
                                         start=True, stop=True)
                        nc.scalar.copy(strip0[:, sc0:sc0 + scw], pA[0:96, 0:scw])
                        nc.scalar.copy(asadT[:, sc0:sc0 + scw], pA[96:120, 0:scw])
                        pB = psp.tile([96, 512], F32, tag="psB")
                        nc.tensor.matmul(pB[:, 0:scw], wgb[:], xr[:, 0:scw],
                                         start=True, stop=True)
                        nc.scalar.copy(strip1[:, sc0:sc0 + scw], pB[:, 0:scw])
                    rowblk = wp.tile([128, 8 * AELEM], F32, tag="rowblk")
                    rb3 = rowblk[:].rearrange("p (g e) -> p g e", e=AELEM)
                    nc.vector.memset(rowblk[:], 0.0)
                    hh0 = rb3[:, 0:gtiles, 16:64].bitcast(BF16)
                    nc.sync.dma_start(hh0, strip0[:, 0:gtiles * 128], transpose=True)
                    hh1 = rb3[:, 0:gtiles, 64:112].bitcast(BF16)
                    nc.sync.dma_start(hh1, strip1[:, 0:gtiles * 128], transpose=True)
                    for g in range(gtiles):
                        pt = psp.tile([128, 24], F32, tag="psC")
                        nc.tensor.transpose(
                            pt[:], asadT[:, g * 128:(g + 1) * 128],
                            ident[0:24, 0:24])
                        nc.scalar.copy(rb3[:, g, 0:12], pt[:, 0:12])
                    c0 = s0 * S + loc
                    nc.sync.dma_start(
                        tab_gat[c0:c0 + cw, :].rearrange("(g p) e -> p g e", p=128),
                        rb3[:, 0:gtiles, :])
                    loc += cw
            barrier()

            # ================= GAT gather =================
            grv = gatrows[:].rearrange("p (b c) -> p b c", c=192)
            drv = v3(denrows, 12)
            for ch in range(NCH):
                gs = [gp.tile([128, CRB * AELEM], F32, tag=f"agbuf{k}",
                              name=f"agbuf{k}_{ch}") for k in range(K)]
                g4s = [t[:].rearrange("p (b e) -> p b e", e=AELEM) for t in gs]
                for k in range(K):
                    base = k * RPAD + ch * CHUNK_ROWS
                    nc.gpsimd.dma_gather(
                        g4s[k], tab_gat[:],
                        idxs[:, base // 16:(base + CHUNK_ROWS) // 16],
                        num_idxs=CHUNK_ROWS, num_idxs_reg=CHUNK_ROWS,
                        elem_size=AELEM, queue_num=k % 4)
                ex = gp.tile([128, K * CRB * 12], F32, tag="ex")
                exv = ex[:].rearrange("p (k b h) -> p k b h", k=K, h=12)
                for k in range(K):
                    nc.vector.tensor_tensor(
                        exv[:, k], g4s[k][:, :, 0:12],
                        adrows3[:, ch * CRB:(ch + 1) * CRB, :], op=OP.add)
                lk = gp.tile([128, K * CRB * 12], F32, tag="lk")
                lkv = lk[:].rearrange("p (k b h) -> p k b h", k=K, h=12)
                nc.scalar.mul(lkv, exv, 0.2)
                nc.vector.tensor_tensor(exv, exv, lkv, op=OP.max)
                nc.scalar.activation(exv, exv, AF.Exp)
                nc.vector.tensor_tensor(lkv[:, 0:2], exv[:, 0:2], exv[:, 2:4], op=OP.add)
                nc.vector.tensor_tensor(
                    drv[:, ch * CRB:(ch + 1) * CRB, :].unsqueeze(1),
                    lkv[:, 0:1], lkv[:, 1:2], op=OP.add)
                exb = gp.tile([128, K * CRB * 12], BF16, tag="exb")
                exbv = exb[:].rearrange("p (k b h) -> p k b h", k=K, h=12)
                nc.vector.tensor_copy(exbv, exv)
                hhs = [g4s[k][:, :, 16:112].bitcast(BF16).rearrange(
                    "p b (h c) -> p b h c", c=HID) for k in range(K)]
                for k in range(K):
                    nc.vector.tensor_tensor(
                        hhs[k], hhs[k],
                        exbv[:, k].unsqueeze(3).broadcast_to([128, CRB, 12, HID]),
                        op=OP.mult)
                hhf = [h.rearrange("p b h c -> p b (h c)") for h in hhs]
                for k in range(2):
                    nc.vector.tensor_tensor(hhf[k], hhf[k], hhf[k + 2], op=OP.add)
                nc.vector.tensor_tensor(
                    grv[:, ch * CRB:(ch + 1) * CRB, :], hhf[0], hhf[1], op=OP.add)
            barrier()

            # ================= GAT epilogue =================
            num = wp.tile([128, SB * 192], F32, tag="num")
            num3 = v3(num, 192)
            combine_rows(grv, num3)
            den = wp.tile([128, SB * 12], F32, tag="den")
            den3 = v3(den, 12)
            combine_rows(drv, den3)
            nc.vector.reciprocal(den3, den3)
            num4 = num3.rearrange("p b (h c) -> p b h c", c=HID)
            nc.vector.tensor_tensor(
                num4, num4,
                den3.unsqueeze(3).broadcast_to([128, SB, 12, HID]), op=OP.mult)
            nc.vector.tensor_tensor(
                num3, num3, bgt[:].unsqueeze(1).broadcast_to([128, SB, 192]),
                op=OP.add)
            for b in range(SB):
                el1 = wp.tile([128, 192], F32, tag="el1", name=f"el1_{b}")
                nc.vector.tensor_scalar_min(el1[:], num3[:, b, :], 0.0)
                nc.scalar.activation(el1[:], el1[:], AF.Exp)
                nc.scalar.activation(num3[:, b, :], num3[:, b, :], AF.Relu)
                nc.vector.tensor_tensor(num3[:, b, :], num3[:, b, :], el1[:],
                                        op=OP.add)
                nc.vector.tensor_scalar_add(num3[:, b, :], num3[:, b, :], -1.0)
            nc.vector.tensor_tensor(
                num3, num3, dinv[:].unsqueeze(2).broadcast_to([128, SB, 192]),
                op=OP.mult)

            # ================= GCN2 =================
            h2n = wp.tile([128, SB * HID], F32, tag="hn")
            project(v3(h2n, HID), num3,
                    [(w2a[:], 0, 96), (w2b[:], 96, 96)], HID)
            ag_and_table(v3(h2n, HID))
            gcn_gather_pass()
            gcn_epilogue(v3(x3n, HID), 16)

            # ================= GCN3 =================
            x3sc = wp.tile([128, SB * HID], F32, tag="x3sc")
            nc.vector.tensor_tensor(
                v3(x3sc, HID), v3(x3n, HID),
                dinv[:].unsqueeze(2).broadcast_to([128, SB, HID]), op=OP.mult)
            h3n = wp.tile([128, SB * HID], F32, tag="hn")
            project(v3(h3n, HID), v3(x3sc, HID), [(w3[:], 0, HID)], HID)
            ag_and_table(v3(h3n, HID))
            gcn_gather_pass()
            gcn_epilogue(v3(x4n, HID), 32, residual=v3(x3n, HID))

            # ================= mean pool + linear =================
            x4m = wp.tile([128, SB * HID], F32, tag="x4m")
            nc.vector.tensor_tensor(
                v3(x4m, HID), v3(x4n, HID),
                mask[:].unsqueeze(2).broadcast_to([128, SB, HID]), op=OP.mult)
            ones = pp.tile([128, 1], F32)
            nc.vector.memset(ones[:], 1.0)
            pool_ps = psp.tile([HID, 1], F32, tag="psB")
            for b in range(SB):
                nc.tensor.matmul(
                    pool_ps[:], v3(x4m, HID)[:, b, :], ones[:],
                    start=(b == 0), stop=(b == SB - 1))
            pool = wp.tile([HID, 1], F32, tag="pool_sb")
            nc.scalar.copy(pool[:], pool_ps[:])
            barrier()
            with tc.tile_critical():
                nc.gpsimd.dma_start(arin[:], pool[:]).then_inc(io_sem, 16)
                st["io"] += 16
                nc.gpsimd.wait_ge(io_sem, st["io"])
                nc.gpsimd.collective_compute(
                    "AllReduce", OP.add, replica_groups=[list(range(NCORES))],
                    ins=[arin[:]], outs=[arout[:]],
                ).then_inc(cc_sem)
                st["cc"] += 1
                nc.gpsimd.wait_ge(cc_sem, st["cc"])
            barrier()
            poolg = wp.tile([HID, 1], F32, tag="poolg")
            nc.sync.dma_start(poolg[:], arout[:])
            nc.scalar.mul(poolg[:], poolg[:], 1.0 / N)
            out_ps = psp.tile([1, OUT], F32, tag="psB")
            nc.tensor.matmul(out_ps[:], poolg[:], wlin[:], start=True, stop=True)
            outt = wp.tile([1, OUT], F32, tag="outt")
            nc.scalar.copy(outt[:], out_ps[:])
            nc.vector.tensor_tensor(outt[:], outt[:], biasb[0:1, 48:48 + OUT],
                                    op=OP.add)
            nc.sync.dma_start(out_ext[:], outt[:])

    nc.compile()
    return nc


# ---------------------------------------------------------------- entry point
def _make_in_maps(inputs, cores, meta):
    x = np.asarray(inputs["x"], np.float32)
    S = meta["SHARDR"]
    W1 = np.asarray(inputs["W1"], np.float32)
    Wg = np.asarray(inputs["Wg"], np.float32)
    att_src = np.asarray(inputs["att_src"], np.float32)
    att_dst = np.asarray(inputs["att_dst"], np.float32)
    W2 = np.asarray(inputs["W2"], np.float32)
    W3 = np.asarray(inputs["W3"], np.float32)
    Wlin = np.asarray(inputs["Wlin"], np.float32)

    Wg3 = Wg.reshape(16, 12, 16)
    Was = np.einsum("khc,hc->kh", Wg3, att_src).astype(np.float32)
    Wad = np.einsum("khc,hc->kh", Wg3, att_dst).astype(np.float32)
    wga = np.concatenate([Wg[:, 0:96], Was, Wad], axis=1)  # [16, 120]
    wgb = np.ascontiguousarray(Wg[:, 96:192])

    biases = np.zeros((1, 256), np.float32)
    biases[0, 0:16] = np.asarray(inputs["b1"], np.float32)
    biases[0, 16:32] = np.asarray(inputs["b2"], np.float32)
    biases[0, 32:48] = np.asarray(inputs["b3"], np.float32)
    biases[0, 48:80] = np.asarray(inputs["blin"], np.float32)
    bg_row = np.asarray(inputs["bg"], np.float32)[None, :]
    padrow = np.zeros((1, AELEM), np.float32)
    padrow[0, 0:12] = -200.0
    ident = np.eye(128, dtype=np.float32)

    xsq = x.reshape(N, F_IN)
    dinv = meta["dinv"]
    in_maps = []
    for c in range(NCORES):
        nl = cores[c]["nodes_local"]
        real = nl >= 0
        xs = np.zeros((S, F_IN), np.float32)
        xs[real] = xsq[nl[real]]
        dv = np.zeros(S, np.float32)
        dv[real] = dinv[nl[real]]
        in_maps.append({
            "xs": xs,
            "dinv": np.ascontiguousarray(_to_pb(dv[:, None], S), dtype=np.float32),
            "mask": np.ascontiguousarray(_to_pb(cores[c]["mask"][:, None], S), dtype=np.float32),
            "idxs": _wrap_idxs(cores[c]["slot_idx"]),
            "w1": W1, "wga": wga, "wgb": wgb,
            "w2a": np.ascontiguousarray(W2[0:96]),
            "w2b": np.ascontiguousarray(W2[96:192]),
            "w3": W3, "wlin": Wlin,
            "biases": biases, "bg": bg_row, "padrow": padrow, "ident": ident,
        })
    return in_maps


def run(inputs, trace=False):
    edge_index = np.asarray(inputs["edge_index"])
    cores, meta = _preprocess(edge_index)
    in_maps = _make_in_maps(inputs, cores, meta)
    nc = _build(meta)
    res = run_bass_kernel_spmd(nc, in_maps, list(range(NCORES)), trace=trace)
    return res


def kernel(**inputs):
    return run(inputs).results[0]["out"]


if __name__ == "__main__":
    import reference
    inputs = reference.setup_inputs()
    inputs = {k: np.asarray(v) for k, v in inputs.items()}
    got = kernel(**inputs)
    exp = np.asarray(reference.reference(**inputs))
    rel = np.abs(got - exp).max() / np.abs(exp).max()
    print("rel err:", rel)


# revision 23
# speedup vs baseline: 1.2052x; 1.0280x over previous
"""Trainium2 Bass kernel for nn_EnhancedGCN (GCN -> GAT -> GCN -> GCN -> mean -> linear).

Strategy (8 NeuronCores, dst-sharded message passing):
- Host: add self loops, in-degree -> dinv (compile-time), relabel nodes
  (core = n % 8, rank grouped by region r = ceil(indeg/K), K=8 slots/row,
  region sizes padded uniform across cores). Slot i = k*RPAD + row holds the
  table index of the k-th in-edge source of `row`; gathered slot i lands at
  SBUF [i%128, i//128], i.e. rows are partition-wrapped and k is a free-dim
  plane -> K-reduce is log-step DVE adds, region combine is block-offset adds.
- Aggregation passes gather from per-core HBM tables via SWDGE dma_gather
  (chunks of 1024 idxs, 4 queues). GCN tables: [TABN+8, 64] f32 rows (16
  used). GAT table: [TABN+8, 128] f32-word rows = [a_src 12 f32 | pad |
  hh 192 bf16 | pad]; the pad row has a_src=-200 so exp() ~ 0.
- a_src/a_dst attention vectors are folded into Wg on the host (Was/Wad);
  softmax max-subtraction is skipped (mathematically identity).
- Between layers: AllGather of [SHARDR, 16] f32 shards; final mean-pool via
  ones-matmul + AllReduce.
"""
import sys

for _p in ("/opt/trn_rl_repo", "/root/.axon_site/_ro/trn_rl_repo"):
    if _p not in sys.path:
        sys.path.append(_p)

import numpy as np

import concourse.bacc as bacc
import concourse.tile as tile
import concourse.mybir as mybir
from concourse.bass_utils import run_bass_kernel_spmd

F32 = mybir.dt.float32
BF16 = mybir.dt.bfloat16
I16 = mybir.dt.int16
AF = mybir.ActivationFunctionType
OP = mybir.AluOpType

NCORES = 8
K = 4            # slots per row
MAXREG = 16
N = 25000
F_IN = 128
HID = 16
OUT = 32
GELEM = 64       # gcn table row, f32 words
AELEM = 128      # gat table row, f32 words
CHUNK_ROWS = 1024


# ---------------------------------------------------------------- host prep
def _preprocess(edge_index):
    src = np.asarray(edge_index[0], np.int64)
    dst = np.asarray(edge_index[1], np.int64)
    loop = np.arange(N, dtype=np.int64)
    src = np.concatenate([src, loop])
    dst = np.concatenate([dst, loop])
    deg = np.bincount(dst, minlength=N)
    dinv = (1.0 / np.sqrt(deg.astype(np.float64))).astype(np.float32)

    core_of = (np.arange(N) % NCORES).astype(np.int64)
    rows_needed = -(-deg // K)
    assert rows_needed.max() <= MAXREG

    reg_nodes = [[None] * (MAXREG + 1) for _ in range(NCORES)]
    for c in range(NCORES):
        mine = np.where(core_of == c)[0]
        for r in range(1, MAXREG + 1):
            sel = mine[rows_needed[mine] == r]
            reg_nodes[c][r] = sel[np.argsort(-deg[sel], kind="stable")]

    reg_size = np.zeros(MAXREG + 1, np.int64)
    for r in range(1, MAXREG + 1):
        m = max(len(reg_nodes[c][r]) for c in range(NCORES))
        reg_size[r] = -(-m // 128) * 128 if m > 0 else 0
    SHARDR = int(reg_size[1:].sum())
    RTOT = int(sum(r * reg_size[r] for r in range(1, MAXREG + 1)))
    RPAD = -(-RTOT // CHUNK_ROWS) * CHUNK_ROWS
    NSLOT = K * RPAD
    TABN = NCORES * SHARDR
    PADROW = TABN
    assert PADROW < 32768, PADROW

    rank_base = np.cumsum([0] + [int(reg_size[r]) for r in range(1, MAXREG + 1)])
    row_base = np.cumsum([0] + [int(r * reg_size[r]) for r in range(1, MAXREG + 1)])

    rank_of = np.full(N, -1, np.int64)
    for c in range(NCORES):
        for ri, r in enumerate(range(1, MAXREG + 1)):
            nodes = reg_nodes[c][r]
            rank_of[nodes] = rank_base[ri] + np.arange(len(nodes))
    tabpos = core_of * SHARDR + rank_of

    cores = []
    for c in range(NCORES):
        slot_idx = np.full(NSLOT, PADROW, np.int64)
        mask = np.zeros(SHARDR, np.float32)
        nodes_local = np.full(SHARDR, -1, np.int64)
        for ri, r in enumerate(range(1, MAXREG + 1)):
            nodes = reg_nodes[c][r]
            mask[rank_base[ri]:rank_base[ri] + len(nodes)] = 1.0
            nodes_local[rank_base[ri]:rank_base[ri] + len(nodes)] = nodes

        emask = core_of[dst] == c
        es, ed = src[emask], dst[emask]
        order = np.argsort(ed, kind="stable")
        es, ed = es[order], ed[order]
        uniq, start_idx = np.unique(ed, return_index=True)
        pos = np.arange(len(ed)) - start_idx[np.searchsorted(uniq, ed)]
        r_of = rows_needed[ed]
        ri_of = r_of - 1
        j = pos // K
        k = pos % K
        rank_in_reg = rank_of[ed] - rank_base[ri_of]
        row = row_base[ri_of] + j * reg_size[r_of] + rank_in_reg
        slot_idx[k * RPAD + row] = tabpos[es]
        cores.append(dict(slot_idx=slot_idx, mask=mask, nodes_local=nodes_local))

    meta = dict(SHARDR=SHARDR, RPAD=RPAD, NSLOT=NSLOT, TABN=TABN, PADROW=PADROW,
                reg_size=reg_size, rank_base=rank_base, row_base=row_base,
                dinv=dinv, tabpos=tabpos)
    return cores, meta


def _wrap_idxs(idx):
    n = len(idx)
    w = idx.reshape(n // 16, 16).T.astype(np.int16)
    return np.tile(w, (8, 1))


def _to_pb(a, S):
    C = a.shape[1] if a.ndim > 1 else 1
    return a.reshape(S // 128, 128, C).transpose(1, 0, 2).reshape(128, -1)


# ---------------------------------------------------------------- kernel build
def _build(meta):
    S, RPAD, NSLOT, TABN = meta["SHARDR"], meta["RPAD"], meta["NSLOT"], meta["TABN"]
    SB = S // 128
    RB = RPAD // 128
    CRB = CHUNK_ROWS // 128
    NCH = RPAD // CHUNK_ROWS
    reg_size, rank_base, row_base = meta["reg_size"], meta["rank_base"], meta["row_base"]

    nc = bacc.Bacc("TRN2", target_bir_lowering=False, debug=False, num_swdge_queues=4)

    xs_in = nc.declare_dram_parameter("xs", [S, F_IN], F32, isOutput=False)
    dinv_in = nc.declare_dram_parameter("dinv", [128, SB], F32, isOutput=False)
    mask_in = nc.declare_dram_parameter("mask", [128, SB], F32, isOutput=False)
    idxs_in = nc.declare_dram_parameter("idxs", [128, NSLOT // 16], I16, isOutput=False)
    w1_in = nc.declare_dram_parameter("w1", [F_IN, HID], F32, isOutput=False)
    wga_in = nc.declare_dram_parameter("wga", [HID, 120], F32, isOutput=False)
    wgb_in = nc.declare_dram_parameter("wgb", [HID, 96], F32, isOutput=False)
    w2a_in = nc.declare_dram_parameter("w2a", [96, HID], F32, isOutput=False)
    w2b_in = nc.declare_dram_parameter("w2b", [96, HID], F32, isOutput=False)
    w3_in = nc.declare_dram_parameter("w3", [HID, HID], F32, isOutput=False)
    wlin_in = nc.declare_dram_parameter("wlin", [HID, OUT], F32, isOutput=False)
    bias_in = nc.declare_dram_parameter("biases", [1, 256], F32, isOutput=False)
    bg_in = nc.declare_dram_parameter("bg", [1, 192], F32, isOutput=False)
    padrow_in = nc.declare_dram_parameter("padrow", [1, AELEM], F32, isOutput=False)
    ident_in = nc.declare_dram_parameter("ident", [128, 128], F32, isOutput=False)
    out_ext = nc.declare_dram_parameter("out", [1, OUT], F32, isOutput=True)

    tab_gcn = nc.dram_tensor("tab_gcn", [TABN + 8, GELEM], F32)
    tab_gat = nc.dram_tensor("tab_gat", [TABN + 8, AELEM], F32)
    agin = nc.dram_tensor("agin", [S, HID], F32)
    agout = nc.dram_tensor("agout", [TABN, HID], F32, addr_space="Shared")
    aginT = nc.dram_tensor("aginT", [32, S], F32)
    agoutT = nc.dram_tensor("agoutT", [NCORES * 32, S], F32, addr_space="Shared")
    arin = nc.dram_tensor("arin", [HID, 1], F32)
    arout = nc.dram_tensor("arout", [HID, 1], F32, addr_space="Shared")

    cc_sem = nc.alloc_semaphore("ccs")
    io_sem = nc.alloc_semaphore("ios")
    st = {"cc": 0, "io": 0}

    with tile.TileContext(nc) as tc:
        with (
            tc.tile_pool(name="persist", bufs=1) as pp,
            tc.tile_pool(name="work", bufs=1) as wp,
            tc.tile_pool(name="gb", bufs=2) as gp,
            tc.tile_pool(name="ps", bufs=2, space="PSUM") as psp,
        ):
            idxs = pp.tile([128, NSLOT // 16], I16)
            nc.sync.dma_start(idxs[:], idxs_in[:])
            dinv = pp.tile([128, SB], F32)
            nc.sync.dma_start(dinv[:], dinv_in[:])
            mask = pp.tile([128, SB], F32)
            nc.sync.dma_start(mask[:], mask_in[:])
            w1 = pp.tile([F_IN, HID], F32)
            nc.sync.dma_start(w1[:], w1_in[:])
            wga = pp.tile([HID, 120], F32)
            nc.sync.dma_start(wga[:], wga_in[:])
            wgb = pp.tile([HID, 96], F32)
            nc.sync.dma_start(wgb[:], wgb_in[:])
            w2a = pp.tile([96, HID], F32)
            nc.sync.dma_start(w2a[:], w2a_in[:])
            w2b = pp.tile([96, HID], F32)
            nc.sync.dma_start(w2b[:], w2b_in[:])
            w3 = pp.tile([HID, HID], F32)
            nc.sync.dma_start(w3[:], w3_in[:])
            wlin = pp.tile([HID, OUT], F32)
            nc.sync.dma_start(wlin[:], wlin_in[:])
            ident = pp.tile([128, 128], F32)
            nc.sync.dma_start(ident[:], ident_in[:])
            bgt = pp.tile([128, 192], F32)
            nc.sync.dma_start(bgt[:], bg_in[:].partition_broadcast(128).squeeze(1))
            biasb = pp.tile([128, 256], F32)
            nc.sync.dma_start(biasb[:], bias_in[:].partition_broadcast(128).squeeze(1))

            nc.sync.dma_start(tab_gat[TABN:TABN + 1, :], padrow_in[:])
            wgab = pp.tile([HID, 120], BF16)
            nc.scalar.copy(wgab[:], wga[:])
            wgbb = pp.tile([HID, 96], BF16)
            nc.scalar.copy(wgbb[:], wgb[:])
            zrow = pp.tile([1, GELEM], F32)
            nc.vector.memset(zrow[:], 0.0)
            nc.sync.dma_start(tab_gcn[TABN:TABN + 1, :], zrow[:])

            x1n = pp.tile([128, SB * HID], F32)
            x3n = pp.tile([128, SB * HID], F32)
            x4n = pp.tile([128, SB * HID], F32)
            gcnrows = pp.tile([128, RB * HID], F32)
            gatrows = pp.tile([128, RB * 192], BF16)
            denrows = pp.tile([128, RB * 12], F32)
            adrows = pp.tile([128, RB * 12], F32)

            def barrier():
                tc.strict_bb_all_engine_barrier()

            def collective(kind, op, src_dram, dst_dram):
                with tc.tile_critical():
                    nc.gpsimd.collective_compute(
                        kind, op, replica_groups=[list(range(NCORES))],
                        ins=[src_dram[:]], outs=[dst_dram[:]],
                    ).then_inc(cc_sem)
                    st["cc"] += 1
                    nc.gpsimd.wait_ge(cc_sem, st["cc"])
                barrier()

            def v3(t, c):
                return t[:].rearrange("p (b c) -> p b c", c=c)

            # per-tile node-major projection: dst[:, b, :] = src[:, b, :] @ Ws
            def project(dst3d, src3d, wlist, cdim):
                """wlist = [(W_ap, src_off, fdim), ...] summed over chunks."""
                for b in range(SB):
                    mm = psp.tile([cdim, 128], F32, tag="psB")
                    for i, (W, off, fd) in enumerate(wlist):
                        tp = psp.tile([fd, 128], F32, tag="psA")
                        nc.tensor.transpose(
                            tp[:], src3d[:, b, off:off + fd], ident[:])
                        tps = wp.tile([128, 128], F32, tag="ptps")
                        nc.scalar.copy(tps[0:fd, :], tp[:])
                        nc.tensor.matmul(mm[:], W, tps[0:fd, :],
                                         start=(i == 0), stop=(i == len(wlist) - 1))
                    mms = wp.tile([cdim, 128], F32, tag="pmms")
                    nc.scalar.copy(mms[:], mm[:])
                    tb = psp.tile([128, cdim], F32, tag="psC")
                    nc.tensor.transpose(tb[:], mms[:], ident[0:cdim, 0:cdim])
                    nc.scalar.copy(dst3d[:, b, :], tb[:])

            def ag_and_table(hn3d):
                nc.sync.dma_start(
                    agin[:].rearrange("(b p) f -> p b f", p=128), hn3d)
                barrier()
                collective("AllGather", OP.bypass, agin, agout)
                nc.sync.dma_start(tab_gcn[0:TABN, 0:HID], agout[:])
                barrier()

            def gcn_gather_pass():
                rows16 = v3(gcnrows, HID)
                for ch in range(NCH):
                    gs = [gp.tile([128, CRB * GELEM], F32, tag=f"gbuf{k}",
                                  name=f"gbuf{k}_{ch}") for k in range(K)]
                    g4s = [t[:].rearrange("p (b e) -> p b e", e=GELEM) for t in gs]
                    for k in range(K):
                        base = k * RPAD + ch * CHUNK_ROWS
                        nc.gpsimd.dma_gather(
                            g4s[k], tab_gcn[:],
                            idxs[:, base // 16:(base + CHUNK_ROWS) // 16],
                            num_idxs=CHUNK_ROWS, num_idxs_reg=CHUNK_ROWS,
                            elem_size=GELEM, queue_num=k % 4)
                    t2 = gp.tile([128, 2 * CRB * HID], F32, tag="gcn2")
                    t2v = t2[:].rearrange("p (k b c) -> p k b c", k=2, c=HID)
                    for k in range(2):
                        nc.vector.tensor_tensor(
                            t2v[:, k], g4s[k][:, :, 0:HID], g4s[k + 2][:, :, 0:HID],
                            op=OP.add)
                    nc.vector.tensor_tensor(
                        rows16[:, ch * CRB:(ch + 1) * CRB, :].unsqueeze(1),
                        t2v[:, 0:1], t2v[:, 1:2], op=OP.add)

            def combine_rows(rows3d, out3d):
                for ri, r in enumerate(range(1, MAXREG + 1)):
                    sz = int(reg_size[r])
                    if sz == 0:
                        continue
                    rb0 = int(row_base[ri]) // 128
                    kb0 = int(rank_base[ri]) // 128
                    nblk = sz // 128
                    dst = out3d[:, kb0:kb0 + nblk, :]
                    if r == 1:
                        nc.vector.tensor_copy(dst, rows3d[:, rb0:rb0 + nblk, :])
                    else:
                        nc.vector.tensor_tensor(
                            dst, rows3d[:, rb0:rb0 + nblk, :],
                            rows3d[:, rb0 + nblk:rb0 + 2 * nblk, :], op=OP.add)
                        for j in range(2, r):
                            nc.vector.tensor_tensor(
                                dst, dst,
                                rows3d[:, rb0 + j * nblk:rb0 + (j + 1) * nblk, :],
                                op=OP.add)

            def gcn_epilogue(xdst3d, bias_off, residual=None):
                agg = wp.tile([128, SB * HID], F32, tag="agg")
                agg3 = v3(agg, HID)
                combine_rows(v3(gcnrows, HID), agg3)
                nc.vector.tensor_tensor(
                    agg3, agg3, dinv[:].unsqueeze(2).broadcast_to([128, SB, HID]),
                    op=OP.mult)
                nc.vector.tensor_tensor(
                    agg3, agg3,
                    biasb[:, bias_off:bias_off + HID].unsqueeze(1).broadcast_to(
                        [128, SB, HID]), op=OP.add)
                nc.scalar.activation(xdst3d, agg3, AF.Relu)
                if residual is not None:
                    nc.vector.tensor_tensor(xdst3d, xdst3d, residual, op=OP.add)

            # ================= GCN1 =================
            xsc = wp.tile([128, SB * F_IN], F32, tag="num")
            nc.sync.dma_start(
                v3(xsc, F_IN), xs_in[:].rearrange("(b p) f -> p b f", p=128))
            nc.vector.tensor_tensor(
                v3(xsc, F_IN), v3(xsc, F_IN),
                dinv[:].unsqueeze(2).broadcast_to([128, SB, F_IN]), op=OP.mult)
            h1n = wp.tile([128, SB * HID], F32, tag="hn")
            project(v3(h1n, HID), v3(xsc, F_IN), [(w1[:], 0, F_IN)], HID)
            ag_and_table(v3(h1n, HID))
            gcn_gather_pass()
            gcn_epilogue(v3(x1n, HID), 0)

            # ================= GAT prep =================
            # per-tile: x1T tile -> aginT cols; adT tile -> adr
            adr = wp.tile([128, SB * 12], F32, tag="adr")
            zc = wp.tile([16, 128], F32, tag="zc")
            nc.vector.memset(zc[:], 0.0)
            for b in range(SB):
                tp = psp.tile([HID, 128], F32, tag="psA")
                nc.tensor.transpose(tp[:], v3(x1n, HID)[:, b, :], ident[:])
                tps = wp.tile([HID, 128], F32, tag="x1tt")
                nc.scalar.copy(tps[:], tp[:])
                nc.sync.dma_start(aginT[0:16, b * 128:(b + 1) * 128], tps[:])
                nc.sync.dma_start(aginT[16:32, b * 128:(b + 1) * 128], zc[:])
                ad_ps = psp.tile([12, 128], F32, tag="psB")
                nc.tensor.matmul(ad_ps[:], wga[:, 108:120], tps[:],
                                 start=True, stop=True)
                ad_sb = wp.tile([12, 128], F32, tag="adsb")
                nc.scalar.copy(ad_sb[:], ad_ps[:])
                tb = psp.tile([128, 12], F32, tag="psC")
                nc.tensor.transpose(tb[:], ad_sb[:], ident[0:12, 0:12])
                nc.scalar.copy(v3(adr, 12)[:, b, :], tb[:])
            barrier()
            adrows3 = v3(adrows, 12)
            adr3 = v3(adr, 12)
            for ri, r in enumerate(range(1, MAXREG + 1)):
                sz = int(reg_size[r])
                if sz == 0:
                    continue
                rb0, kb0, nblk = int(row_base[ri]) // 128, int(rank_base[ri]) // 128, sz // 128
                for j in range(r):
                    nc.vector.tensor_copy(
                        adrows3[:, rb0 + j * nblk:rb0 + (j + 1) * nblk, :],
                        adr3[:, kb0:kb0 + nblk, :])
            barrier()
            collective("AllGather", OP.bypass, aginT, agoutT)
            # build tab_gat per shard in column-chunks of <=1024 nodes
            for s0 in range(NCORES):
                loc = 0
                while loc < S:
                    cw = min(1024, S - loc)
                    gtiles = cw // 128
                    strip0 = wp.tile([96, 1024], BF16, tag="strip0")
                    strip1 = wp.tile([96, 1024], BF16, tag="strip1")
                    asadT = wp.tile([24, 1024], F32, tag="asadT")
                    nsub = -(-cw // 512)
                    for ci in range(nsub):
                        sc0 = ci * 512
                        scw = min(512, cw - sc0)
                        xr = wp.tile([16, 512], F32, tag="xr")
                        nc.sync.dma_start(
                            xr[:, 0:scw],
                            agoutT[32 * s0:32 * s0 + 16, loc + sc0:loc + sc0 + scw])
                        xrb = wp.tile([16, 512], BF16, tag="xrb")
                        nc.vector.tensor_copy(xrb[:, 0:scw], xr[:, 0:scw])
                        pA = psp.tile([120, 512], F32, tag="psA")
                        nc.tensor.matmul(pA[:, 0:scw], wgab[:], xrb[:, 0:scw],
                                         start=True, stop=True)
                        nc.scalar.copy(strip0[:, sc0:sc0 + scw], pA[0:96, 0:scw])
                        nc.scalar.copy(asadT[:, sc0:sc0 + scw], pA[96:120, 0:scw])
                        pB = psp.tile([96, 512], F32, tag="psB")
                        nc.tensor.matmul(pB[:, 0:scw], wgbb[:], xrb[:, 0:scw],
                                         start=True, stop=True)
                        nc.scalar.copy(strip1[:, sc0:sc0 + scw], pB[:, 0:scw])
                    rowblk = wp.tile([128, 8 * AELEM], F32, tag="rowblk")
                    rb3 = rowblk[:].rearrange("p (g e) -> p g e", e=AELEM)
                    nc.vector.memset(rowblk[:], 0.0)
                    hh0 = rb3[:, 0:gtiles, 16:64].bitcast(BF16)
                    nc.sync.dma_start(hh0, strip0[:, 0:gtiles * 128], transpose=True)
                    hh1 = rb3[:, 0:gtiles, 64:112].bitcast(BF16)
                    nc.sync.dma_start(hh1, strip1[:, 0:gtiles * 128], transpose=True)
                    for g in range(gtiles):
                        pt = psp.tile([128, 24], F32, tag="psC")
                        nc.tensor.transpose(
                            pt[:], asadT[:, g * 128:(g + 1) * 128],
                            ident[0:24, 0:24])
                        nc.scalar.copy(rb3[:, g, 0:12], pt[:, 0:12])
                    c0 = s0 * S + loc
                    nc.sync.dma_start(
                        tab_gat[c0:c0 + cw, :].rearrange("(g p) e -> p g e", p=128),
                        rb3[:, 0:gtiles, :])
                    loc += cw
            barrier()

            # ================= GAT gather =================
            grv = gatrows[:].rearrange("p (b c) -> p b c", c=192)
            drv = v3(denrows, 12)
            for ch in range(NCH):
                gs = [gp.tile([128, CRB * AELEM], F32, tag=f"agbuf{k}",
                              name=f"agbuf{k}_{ch}") for k in range(K)]
                g4s = [t[:].rearrange("p (b e) -> p b e", e=AELEM) for t in gs]
                for k in range(K):
                    base = k * RPAD + ch * CHUNK_ROWS
                    nc.gpsimd.dma_gather(
                        g4s[k], tab_gat[:],
                        idxs[:, base // 16:(base + CHUNK_ROWS) // 16],
                        num_idxs=CHUNK_ROWS, num_idxs_reg=CHUNK_ROWS,
                        elem_size=AELEM, queue_num=k % 4)
                ex = gp.tile([128, K * CRB * 12], F32, tag="ex")
                exv = ex[:].rearrange("p (k b h) -> p k b h", k=K, h=12)
                for k in range(K):
                    nc.vector.tensor_tensor(
                        exv[:, k], g4s[k][:, :, 0:12],
                        adrows3[:, ch * CRB:(ch + 1) * CRB, :], op=OP.add)
                lk = gp.tile([128, K * CRB * 12], F32, tag="lk")
                lkv = lk[:].rearrange("p (k b h) -> p k b h", k=K, h=12)
                nc.scalar.mul(lkv, exv, 0.2)
                nc.vector.tensor_tensor(exv, exv, lkv, op=OP.max)
                nc.scalar.activation(exv, exv, AF.Exp)
                nc.vector.tensor_tensor(lkv[:, 0:2], exv[:, 0:2], exv[:, 2:4], op=OP.add)
                nc.vector.tensor_tensor(
                    drv[:, ch * CRB:(ch + 1) * CRB, :].unsqueeze(1),
                    lkv[:, 0:1], lkv[:, 1:2], op=OP.add)
                exb = gp.tile([128, K * CRB * 12], BF16, tag="exb")
                exbv = exb[:].rearrange("p (k b h) -> p k b h", k=K, h=12)
                nc.vector.tensor_copy(exbv, exv)
                hhs = [g4s[k][:, :, 16:112].bitcast(BF16).rearrange(
                    "p b (h c) -> p b h c", c=HID) for k in range(K)]
                for k in range(K):
                    nc.vector.tensor_tensor(
                        hhs[k], hhs[k],
                        exbv[:, k].unsqueeze(3).broadcast_to([128, CRB, 12, HID]),
                        op=OP.mult)
                hhf = [h.rearrange("p b h c -> p b (h c)") for h in hhs]
                for k in range(2):
                    nc.vector.tensor_tensor(hhf[k], hhf[k], hhf[k + 2], op=OP.add)
                nc.vector.tensor_tensor(
                    grv[:, ch * CRB:(ch + 1) * CRB, :], hhf[0], hhf[1], op=OP.add)
            barrier()

            # ================= GAT epilogue =================
            num = wp.tile([128, SB * 192], F32, tag="num")
            num3 = v3(num, 192)
            combine_rows(grv, num3)
            den = wp.tile([128, SB * 12], F32, tag="den")
            den3 = v3(den, 12)
            combine_rows(drv, den3)
            nc.vector.reciprocal(den3, den3)
            num4 = num3.rearrange("p b (h c) -> p b h c", c=HID)
            nc.vector.tensor_tensor(
                num4, num4,
                den3.unsqueeze(3).broadcast_to([128, SB, 12, HID]), op=OP.mult)
            nc.vector.tensor_tensor(
                num3, num3, bgt[:].unsqueeze(1).broadcast_to([128, SB, 192]),
                op=OP.add)
            for b in range(SB):
                el1 = wp.tile([128, 192], F32, tag="el1", name=f"el1_{b}")
                nc.vector.tensor_scalar_min(el1[:], num3[:, b, :], 0.0)
                nc.scalar.activation(el1[:], el1[:], AF.Exp)
                nc.scalar.activation(num3[:, b, :], num3[:, b, :], AF.Relu)
                nc.vector.tensor_tensor(num3[:, b, :], num3[:, b, :], el1[:],
                                        op=OP.add)
                nc.vector.tensor_scalar_add(num3[:, b, :], num3[:, b, :], -1.0)
            nc.vector.tensor_tensor(
                num3, num3, dinv[:].unsqueeze(2).broadcast_to([128, SB, 192]),
                op=OP.mult)

            # ================= GCN2 =================
            h2n = wp.tile([128, SB * HID], F32, tag="hn")
            project(v3(h2n, HID), num3,
                    [(w2a[:], 0, 96), (w2b[:], 96, 96)], HID)
            ag_and_table(v3(h2n, HID))
            gcn_gather_pass()
            gcn_epilogue(v3(x3n, HID), 16)

            # ================= GCN3 =================
            x3sc = wp.tile([128, SB * HID], F32, tag="x3sc")
            nc.vector.tensor_tensor(
                v3(x3sc, HID), v3(x3n, HID),
                dinv[:].unsqueeze(2).broadcast_to([128, SB, HID]), op=OP.mult)
            h3n = wp.tile([128, SB * HID], F32, tag="hn")
            project(v3(h3n, HID), v3(x3sc, HID), [(w3[:], 0, HID)], HID)
            ag_and_table(v3(h3n, HID))
            gcn_gather_pass()
            gcn_epilogue(v3(x4n, HID), 32, residual=v3(x3n, HID))

            # ================= mean pool + linear =================
            x4m = wp.tile([128, SB * HID], F32, tag="x4m")
            nc.vector.tensor_tensor(
                v3(x4m, HID), v3(x4n, HID),
                mask[:].unsqueeze(2).broadcast_to([128, SB, HID]), op=OP.mult)
            ones = pp.tile([128, 1], F32)
            nc.vector.memset(ones[:], 1.0)
            pool_ps = psp.tile([HID, 1], F32, tag="psB")
            for b in range(SB):
                nc.tensor.matmul(
                    pool_ps[:], v3(x4m, HID)[:, b, :], ones[:],
                    start=(b == 0), stop=(b == SB - 1))
            pool = wp.tile([HID, 1], F32, tag="pool_sb")
            nc.scalar.copy(pool[:], pool_ps[:])
            barrier()
            with tc.tile_critical():
                nc.gpsimd.dma_start(arin[:], pool[:]).then_inc(io_sem, 16)
                st["io"] += 16
                nc.gpsimd.wait_ge(io_sem, st["io"])
                nc.gpsimd.collective_compute(
                    "AllReduce", OP.add, replica_groups=[list(range(NCORES))],
                    ins=[arin[:]], outs=[arout[:]],
                ).then_inc(cc_sem)
                st["cc"] += 1
                nc.gpsimd.wait_ge(cc_sem, st["cc"])
            barrier()
            poolg = wp.tile([HID, 1], F32, tag="poolg")
            nc.sync.dma_start(poolg[:], arout[:])
            nc.scalar.mul(poolg[:], poolg[:], 1.0 / N)
            out_ps = psp.tile([1, OUT], F32, tag="psB")
            nc.tensor.matmul(out_ps[:], poolg[:], wlin[:], start=True, stop=True)
            outt = wp.tile([1, OUT], F32, tag="outt")
            nc.scalar.copy(outt[:], out_ps[:])
            nc.vector.tensor_tensor(outt[:], outt[:], biasb[0:1, 48:48 + OUT],
                                    op=OP.add)
            nc.sync.dma_start(out_ext[:], outt[:])

    nc.compile()
    return nc


# ---------------------------------------------------------------- entry point
def _make_in_maps(inputs, cores, meta):
    x = np.asarray(inputs["x"], np.float32)
    S = meta["SHARDR"]
    W1 = np.asarray(inputs["W1"], np.float32)
    Wg = np.asarray(inputs["Wg"], np.float32)
    att_src = np.asarray(inputs["att_src"], np.float32)
    att_dst = np.asarray(inputs["att_dst"], np.float32)
    W2 = np.asarray(inputs["W2"], np.float32)
    W3 = np.asarray(inputs["W3"], np.float32)
    Wlin = np.asarray(inputs["Wlin"], np.float32)

    Wg3 = Wg.reshape(16, 12, 16)
    Was = np.einsum("khc,hc->kh", Wg3, att_src).astype(np.float32)
    Wad = np.einsum("khc,hc->kh", Wg3, att_dst).astype(np.float32)
    wga = np.concatenate([Wg[:, 0:96], Was, Wad], axis=1)  # [16, 120]
    wgb = np.ascontiguousarray(Wg[:, 96:192])

    biases = np.zeros((1, 256), np.float32)
    biases[0, 0:16] = np.asarray(inputs["b1"], np.float32)
    biases[0, 16:32] = np.asarray(inputs["b2"], np.float32)
    biases[0, 32:48] = np.asarray(inputs["b3"], np.float32)
    biases[0, 48:80] = np.asarray(inputs["blin"], np.float32)
    bg_row = np.asarray(inputs["bg"], np.float32)[None, :]
    padrow = np.zeros((1, AELEM), np.float32)
    padrow[0, 0:12] = -200.0
    ident = np.eye(128, dtype=np.float32)

    xsq = x.reshape(N, F_IN)
    dinv = meta["dinv"]
    in_maps = []
    for c in range(NCORES):
        nl = cores[c]["nodes_local"]
        real = nl >= 0
        xs = np.zeros((S, F_IN), np.float32)
        xs[real] = xsq[nl[real]]
        dv = np.zeros(S, np.float32)
        dv[real] = dinv[nl[real]]
        in_maps.append({
            "xs": xs,
            "dinv": np.ascontiguousarray(_to_pb(dv[:, None], S), dtype=np.float32),
            "mask": np.ascontiguousarray(_to_pb(cores[c]["mask"][:, None], S), dtype=np.float32),
            "idxs": _wrap_idxs(cores[c]["slot_idx"]),
            "w1": W1, "wga": wga, "wgb": wgb,
            "w2a": np.ascontiguousarray(W2[0:96]),
            "w2b": np.ascontiguousarray(W2[96:192]),
            "w3": W3, "wlin": Wlin,
            "biases": biases, "bg": bg_row, "padrow": padrow, "ident": ident,
        })
    return in_maps


def run(inputs, trace=False):
    edge_index = np.asarray(inputs["edge_index"])
    cores, meta = _preprocess(edge_index)
    in_maps = _make_in_maps(inputs, cores, meta)
    nc = _build(meta)
    res = run_bass_kernel_spmd(nc, in_maps, list(range(NCORES)), trace=trace)
    return res


def kernel(**inputs):
    return run(inputs).results[0]["out"]


if __name__ == "__main__":
    import reference
    inputs = reference.setup_inputs()
    inputs = {k: np.asarray(v) for k, v in inputs.items()}
    got = kernel(**inputs)
    exp = np.asarray(reference.reference(**inputs))
    rel = np.abs(got - exp).max() / np.abs(exp).max()
    print("rel err:", rel)
